# revision 1
# baseline (speedup 1.0000x reference)
"""Multi-head attention (B=4, S=2048, D=512, H=8) on 8 trn2 NeuronCores.

Sharding: core c handles batch b = c//2 and head-group g = c%2 (4 heads,
256 of the 512 model dims). Each core computes its 4 heads' attention and
a partial out-projection [2048, 512]; the host sums the two partials per
batch and adds the output bias.

Device kernel per core (all matmuls bf16 -> f32 PSUM):
  1. QKV projections from pre-transposed xT [512, 2048]:
       Q^T/K^T stored per head, zero-padded from 64 to 128 rows so the
       attention matmuls contract over K=128 (a K=64 matmul leaves half
       the PE array inactive and the HAM clock-gate then never grants
       2.4 GHz; the zero rows are numerically inert).
       V [128, 512] per seq-tile with 64 all-ones columns per head
       (injected via the bias) so the P@V matmul emits the softmax
       row-sum pre-replicated on its partitions 64-127 and runs the
       full M=128 array. wq/bq are pre-scaled by 1/8 on the host.
  2. Per (q-block, head), flash-style: S^T tile [128, 1024] = K_h^T.Q_h,
     exp on ScalarE (PSUM -> SBUF bf16, double-buffered, software
     pipelined), P^T accumulated into O^T [128, 512] over 16 k-tiles.
     ScalarE (exp) is the saturated engine; everything else (V
     projection, heads 2-3's Q/K projection, normalize, out-projection)
     rides a priority work queue drained into the loop's slack.
  3. Normalize per q-tile: DVE reciprocal of the replicated row-sum
     block + same-base multiply, deferred one block so its DVE burst
     never lands on a block boundary.
No max-subtraction in softmax: scores are O(1) by construction, exp is
safe, and the reference softmax is shift-invariant.
"""

import numpy as np
import ml_dtypes

import concourse.bacc as bacc
import concourse.mybir as mybir
from concourse.tile import TileContext
from concourse.bass_utils import run_bass_kernel_spmd

BF16 = mybir.dt.bfloat16
F32 = mybir.dt.float32
AF = mybir.ActivationFunctionType
ALU = mybir.AluOpType

B, S, D = 4, 2048, 512
H_CORE, HD = 4, 64          # heads per core, head dim
DHC = H_CORE * HD           # 256 dims per core
VW = H_CORE * 2 * HD        # 512: V augmented with 64 ones-columns per head
N_CORES = 8

_CACHE = {}


def build_nc():
    nc = bacc.Bacc("TRN2", target_bir_lowering=False, debug=False,
                   num_devices=N_CORES)

    xT_d = nc.declare_dram_parameter("xT", [D, S], BF16, isOutput=False)
    wq_d = nc.declare_dram_parameter("wq", [D, DHC], BF16, isOutput=False)
    wk_d = nc.declare_dram_parameter("wk", [D, DHC], BF16, isOutput=False)
    wv_d = nc.declare_dram_parameter("wv", [D, VW], BF16, isOutput=False)
    wo_d = nc.declare_dram_parameter("wo", [DHC, D], BF16, isOutput=False)
    bq_d = nc.declare_dram_parameter("bq", [DHC, 1], F32, isOutput=False)
    bk_d = nc.declare_dram_parameter("bk", [DHC, 1], F32, isOutput=False)
    bvb_d = nc.declare_dram_parameter("bvb", [128, VW], F32, isOutput=False)
    out_d = nc.declare_dram_parameter("out", [S, D], F32, isOutput=True)

    NQB = 2          # q blocks of 1024
    QB = 1024
    NKT = S // 128   # 16 k tiles

    with TileContext(nc, num_cores=N_CORES) as tc:
        with (
            tc.tile_pool(name="persist", bufs=1) as pp,
            tc.tile_pool(name="pt_pool", bufs=3) as ptp,
            tc.tile_pool(name="rs_pool", bufs=2) as rsp,
            tc.tile_pool(name="ob_pool", bufs=3) as obp,
        ):
            # preload the exp ACT table before anything else: the first
            # real exp otherwise pays a ~2.7us table load that stalls the
            # whole pipeline
            scr = pp.tile([1, 8], F32, tag="scr", name="scr")
            nc.vector.memset(scr[:], 0.0)
            nc.scalar.activation(scr[:], scr[:], AF.Exp)

            # ---- load inputs (Q/K-proj operands first) ----
            xT = [pp.tile([128, S], BF16, tag=f"xT{i}", name=f"xT{i}")
                  for i in range(4)]
            wq = [pp.tile([128, DHC], BF16, tag=f"wq{i}", name=f"wq{i}")
                  for i in range(4)]
            wk = [pp.tile([128, DHC], BF16, tag=f"wk{i}", name=f"wk{i}")
                  for i in range(4)]
            wv = [pp.tile([128, VW], BF16, tag=f"wv{i}", name=f"wv{i}")
                  for i in range(4)]
            bq = [pp.tile([128, 1], F32, tag=f"bq{p}", name=f"bq{p}")
                  for p in range(2)]
            bk = [pp.tile([128, 1], F32, tag=f"bk{p}", name=f"bk{p}")
                  for p in range(2)]
            bvb = pp.tile([128, VW], F32, tag="bvb")
            for i in range(4):
                nc.sync.dma_start(out=xT[i][:], in_=xT_d[128 * i:128 * (i + 1), :])
                nc.sync.dma_start(out=wq[i][:], in_=wq_d[128 * i:128 * (i + 1), :])
                nc.sync.dma_start(out=wk[i][:], in_=wk_d[128 * i:128 * (i + 1), :])
                nc.sync.dma_start(out=wv[i][:], in_=wv_d[128 * i:128 * (i + 1), :])
            nc.sync.dma_start(out=bvb[:], in_=bvb_d[:])
            for p in range(2):
                nc.sync.dma_start(out=bq[p][:], in_=bq_d[128 * p:128 * (p + 1), :])
                nc.sync.dma_start(out=bk[p][:], in_=bk_d[128 * p:128 * (p + 1), :])
            wo = [pp.tile([128, D], BF16, tag=f"wo{p}", name=f"wo{p}")
                  for p in range(2)]
            for p in range(2):
                nc.sync.dma_start(out=wo[p][:], in_=wo_d[128 * p:128 * (p + 1), :])

            QTh = [pp.tile([128, S], BF16, tag=f"QTh{h}", name=f"QTh{h}")
                   for h in range(H_CORE)]
            KTh = [pp.tile([128, S], BF16, tag=f"KTh{h}", name=f"KTh{h}")
                   for h in range(H_CORE)]
            OT = [pp.tile([128, S], BF16, tag=f"OT{p}", name=f"OT{p}")
                  for p in range(2)]
            V = [pp.tile([128, VW], BF16, tag=f"V{st}", name=f"V{st}")
                 for st in range(NKT)]
            for h in range(H_CORE):
                nc.vector.memset(QTh[h][64:128, :], 0.0)
                nc.vector.memset(KTh[h][64:128, :], 0.0)


            # ---- attention + priority work queue ----
            # queue items: (min_kt, cost, fn); a per-step budget of 2 is
            # drained smallest-min_kt-first into the PE slack
            with (
                tc.tile_pool(name="s_ps", bufs=2, space="PSUM") as sps,
                tc.tile_pool(name="o_ps", bufs=2, space="PSUM") as ops,
                tc.tile_pool(name="aux_ps", bufs=2, space="PSUM") as axp,
            ):
                aux = []
                fast = []       # normalize closures: jump the main queue

                def v_proj(st):
                    def run():
                        ps = axp.tile([128, 512], F32, tag="aux", name="aux")
                        for din in range(4):
                            nc.tensor.matmul(
                                ps[:, 0:VW],
                                xT[din][:, 128 * st:128 * (st + 1)],
                                wv[din][:],
                                start=(din == 0), stop=(din == 3),
                            )
                        nc.vector.tensor_tensor(
                            out=V[st][:], in0=ps[:, 0:VW], in1=bvb[:],
                            op=ALU.add)
                    return run

                def qk_proj(w_sb, b_sb, dst, wcol, st):
                    def run():
                        ps = axp.tile([128, 512], F32, tag="aux", name="aux")
                        for din in range(4):
                            nc.tensor.matmul(
                                ps[:],
                                w_sb[din][:, 128 * wcol:128 * (wcol + 1)],
                                xT[din][:, 512 * st:512 * (st + 1)],
                                start=(din == 0), stop=(din == 3),
                            )
                        for m in range(2):
                            nc.vector.tensor_scalar(
                                out=dst[2 * wcol + m][0:64,
                                                      512 * st:512 * (st + 1)],
                                in0=ps[64 * m:64 * (m + 1), :],
                                scalar1=b_sb[wcol][64 * m:64 * (m + 1), :],
                                scalar2=None, op0=ALU.add,
                            )
                    return run

                def outproj(st, pool, copy_engine=None):
                    def run():
                        ps = pool.tile([128, 512], F32, tag="aux",
                                       name="aux")
                        for p in range(2):
                            nc.tensor.matmul(
                                ps[:],
                                OT[p][:, 128 * st:128 * (st + 1)],
                                wo[p][:],
                                start=(p == 0), stop=(p == 1),
                            )
                        ob = obp.tile([128, D], F32, tag="ob", name="ob")
                        if copy_engine is None:
                            nc.vector.tensor_copy(ob[:], ps[:])
                        else:
                            copy_engine.copy(ob[:], ps[:])
                        nc.sync.dma_start(
                            out=out_d[128 * st:128 * (st + 1), :], in_=ob[:])
                    return run

                # inline: Q/K dout-tile-0 for seq cols 0-1023 (all that
                # blocks 0-1's S matmuls need for q-block 0 / kt 0-7) and
                # V[0..2]; everything else is deadline-ordered in the queue
                for st in range(2):
                    qk_proj(wq, bq, QTh, 0, st)()
                    qk_proj(wk, bk, KTh, 0, st)()
                v_proj(0)()
                v_proj(1)()
                v_proj(2)()
                # block 0 pops (one item per step, kt 1..15); deadlines:
                # V[st] by step st+1, K cols for S(kt) by step kt-1
                aux.append((1, 2, v_proj(3)))
                aux.append((2, 2, v_proj(4)))
                aux.append((3, 2, qk_proj(wk, bk, KTh, 0, 2)))
                for st in range(5, 9):
                    aux.append((st - 1, 2, v_proj(st)))
                aux.append((8, 2, qk_proj(wk, bk, KTh, 0, 3)))
                for st in range(9, NKT):
                    aux.append((st, 2, v_proj(st)))
                # block 1+: heads 2-3's projection, then Q cols for q-block 1
                for st in range(4):
                    aux.append((1, 2, qk_proj(wq, bq, QTh, 1, st)))
                    aux.append((1, 2, qk_proj(wk, bk, KTh, 1, st)))
                for st in (2, 3):
                    aux.append((1, 2, qk_proj(wq, bq, QTh, 0, st)))

                def block(h, qb):
                    p, m = divmod(h, 2)
                    r0, r1 = 64 * m, 64 * (m + 1)
                    q0 = QB * qb
                    o_acc = []

                    def s_mms(kt):
                        stile = sps.tile([128, QB], F32, tag="s", name="s")
                        for qt in range(2):
                            nc.tensor.matmul(
                                stile[:, 512 * qt:512 * (qt + 1)],
                                KTh[h][:, 128 * kt:128 * (kt + 1)],
                                QTh[h][:, q0 + 512 * qt:q0 + 512 * (qt + 1)],
                                start=True, stop=True,
                            )
                        return stile

                    def exp_pv(kt, stile):
                        pt = ptp.tile([128, QB], BF16, tag="pt", name="pt")
                        nc.scalar.activation(pt[:], stile[:], AF.Exp)
                        for qt in range(2):
                            nc.tensor.matmul(
                                o_acc[qt][:],
                                V[kt][:, 2 * HD * h:2 * HD * (h + 1)],
                                pt[:, 512 * qt:512 * (qt + 1)],
                                start=(kt == 0), stop=(kt == NKT - 1),
                            )

                    prev = s_mms(0)
                    for kt in range(1, NKT):
                        cur = s_mms(kt)
                        if not o_acc:
                            o_acc.extend(
                                ops.tile([128, 512], F32, tag="o",
                                         name="o_acc") for _ in range(2))
                        # fast queue first (normalizes are independent
                        # of everything queued), then strict FIFO: enqueue
                        # order encodes producer -> consumer program order
                        budget = 2
                        while fast and budget > 0 and kt >= 4:
                            fast.pop(0)()
                            budget -= 1
                        while (aux and aux[0][0] <= kt
                               and aux[0][1] <= budget):
                            _, c, fn = aux.pop(0)
                            fn()
                            budget -= c
                        exp_pv(kt - 1, prev)
                        prev = cur
                    exp_pv(NKT - 1, prev)

                    # free the o_acc PSUM slots with one fast DVE copy
                    # each; the slow reciprocal+multiply is deferred into
                    # the next block via the queue
                    osb = []
                    for qt in range(2):
                        t = rsp.tile([128, 512], F32, tag="osb", name="osb")
                        nc.vector.tensor_copy(t[:], o_acc[qt][:])
                        osb.append(t)

                    def normalize(qt):
                        def run():
                            recB = rsp.tile([HD, 512], F32, tag="recB",
                                            name="recB")
                            nc.vector.reciprocal(
                                recB[:], osb[qt][HD:2 * HD, :])
                            nc.vector.tensor_tensor(
                                out=OT[p][r0:r1,
                                          q0 + 512 * qt:q0 + 512 * (qt + 1)],
                                in0=osb[qt][0:HD, :], in1=recB[:],
                                op=ALU.mult,
                            )
                        return run
                    return [normalize(0), normalize(1)], (osb, p, r0, r1, q0)

                last_norm_parts = None
                for bi, (qb, h) in enumerate(
                        (qb, h) for qb in range(NQB)
                        for h in range(H_CORE)):
                    norms, parts = block(h, qb)
                    if (qb, h) == (NQB - 1, H_CORE - 1):
                        last_norm_parts = parts
                    else:
                        fast.extend(norms)
                    # qb0's out-projection pops two blocks after its last
                    # head so the deferred normalizes (behind the DVE
                    # reciprocal queue) are guaranteed to have landed
                    if bi == 4:
                        for st in range(8):
                            aux.append((1, 1, outproj(st, axp)))
                tail_sts = list(range(8, 16))
                leftovers = fast + [fn for _, _, fn in aux]

            # tail: drain with a deeper pool; interleave the last block's
            # normalize with the out-projections that don't depend on it
            with tc.tile_pool(name="tail_ps", bufs=4, space="PSUM") as tlp:
                for fn in leftovers:
                    fn()
                # chunk the last normalize per 128-col piece so each
                # final out-projection waits only on its own columns
                for qt, sts in ((0, tail_sts[:4]), (1, tail_sts[4:])):
                    osb, p, r0, r1, q0 = last_norm_parts
                    for j, st in enumerate(sts):
                        c0, c1 = 128 * j, 128 * (j + 1)
                        recC = rsp.tile([HD, 128], F32, tag="recC",
                                        name="recC")
                        nc.vector.reciprocal(
                            recC[:], osb[qt][HD:2 * HD, c0:c1])
                        nc.vector.tensor_tensor(
                            out=OT[p][r0:r1,
                                      q0 + 512 * qt + c0:q0 + 512 * qt + c1],
                            in0=osb[qt][0:HD, c0:c1], in1=recC[:],
                            op=ALU.mult,
                        )
                        outproj(st, tlp)()

    nc.compile()
    return nc


def _prep_core(x, wq, bq, wk, bk, wv, bv, wo, bo, b, g):
    hs = slice(DHC * g, DHC * (g + 1))
    xT = np.ascontiguousarray(x[b].T).astype(ml_dtypes.bfloat16)
    wq_c = (wq[:, hs] / 8.0).astype(ml_dtypes.bfloat16)
    bq_c = (bq[hs] / 8.0).astype(np.float32).reshape(DHC, 1)
    wk_c = wk[:, hs].astype(ml_dtypes.bfloat16)
    bk_c = bk[hs].astype(np.float32).reshape(DHC, 1)
    wv_aug = np.zeros((D, VW), np.float32)
    bvb = np.zeros((128, VW), np.float32)
    for h in range(H_CORE):
        c0 = 2 * HD * h
        wv_aug[:, c0:c0 + HD] = wv[:, DHC * g + HD * h:DHC * g + HD * (h + 1)]
        bvb[:, c0:c0 + HD] = bv[DHC * g + HD * h:DHC * g + HD * (h + 1)][None, :]
        bvb[:, c0 + HD:c0 + 2 * HD] = 1.0
    wo_c = wo[hs, :].astype(ml_dtypes.bfloat16)
    return {
        "xT": xT,
        "wq": wq_c, "bq": bq_c,
        "wk": wk_c, "bk": bk_c,
        "wv": wv_aug.astype(ml_dtypes.bfloat16), "bvb": bvb,
        "wo": wo_c,
    }


def kernel(x, wq, bq, wk, bk, wv, bv, wo, bo):
    x = np.asarray(x, np.float32)
    wq, bq = np.asarray(wq, np.float32), np.asarray(bq, np.float32)
    wk, bk = np.asarray(wk, np.float32), np.asarray(bk, np.float32)
    wv, bv = np.asarray(wv, np.float32), np.asarray(bv, np.float32)
    wo, bo = np.asarray(wo, np.float32), np.asarray(bo, np.float32)

    if "nc" not in _CACHE:
        _CACHE["nc"] = build_nc()
    nc = _CACHE["nc"]

    in_maps = []
    for c in range(N_CORES):
        b, g = divmod(c, 2)
        in_maps.append(_prep_core(x, wq, bq, wk, bk, wv, bv, wo, bo, b, g))

    res = run_bass_kernel_spmd(nc, in_maps, list(range(N_CORES)))

    out = np.empty((B, S, D), np.float32)
    for b in range(B):
        out[b] = (res.results[2 * b]["out"] + res.results[2 * b + 1]["out"]
                  + bo[None, :])
    return out



# revision 9
# speedup vs baseline: 1.0948x; 1.0948x over previous
"""Multi-head attention (B=4, S=2048, D=512, H=8) on 8 trn2 NeuronCores.

Sharding: core c handles batch b = c//2 and head-group g = c%2 (4 heads,
256 of the 512 model dims). Each core computes its 4 heads' attention and
a partial out-projection [2048, 512]; the host sums the two partials per
batch and adds the output bias.

Device kernel per core (all matmuls bf16 -> f32 PSUM):
  1. QKV projections from pre-transposed xT [512, 2048]:
       Q^T/K^T stored per head, zero-padded from 64 to 128 rows so the
       attention matmuls contract over K=128 (a K=64 matmul leaves half
       the PE array inactive and the HAM clock-gate then never grants
       2.4 GHz; the zero rows are numerically inert).
       V [128, 512] per seq-tile with 64 all-ones columns per head
       (injected via the bias) so the P@V matmul emits the softmax
       row-sum pre-replicated on its partitions 64-127 and runs the
       full M=128 array. wq/bq are pre-scaled by 1/8 on the host.
       All inputs arrive as a handful of merged [128, N] DMAs (host
       packs the 128-row tiles side by side) so the Sync queue isn't
       serialized on ~40 descriptor issues at startup.
  2. Per (q-block, head), flash-style: S^T tile [128, 1024] = K_h^T.Q_h,
     exp on ScalarE (PSUM -> SBUF bf16, double-buffered, software
     pipelined), P^T accumulated into O^T [128, 512] over 16 k-tiles.
     ScalarE (exp) is the saturated engine; everything else (V
     projection, heads 2-3's Q/K projection, normalize, out-projection)
     rides a priority work queue drained into the loop's slack.
  3. Normalize per q-tile: DVE approximate reciprocal (~18-bit, 5x
     faster than the exact iterative divide) of the replicated row-sum
     block + same-base multiply, deferred one block so its DVE burst
     never lands on a block boundary.
No max-subtraction in softmax: scores are O(1) by construction, exp is
safe, and the reference softmax is shift-invariant.
"""

import numpy as np
import ml_dtypes

import concourse.bacc as bacc
import concourse.mybir as mybir
from concourse.tile import TileContext
from concourse.bass_utils import run_bass_kernel_spmd

BF16 = mybir.dt.bfloat16
F32 = mybir.dt.float32
AF = mybir.ActivationFunctionType
ALU = mybir.AluOpType

B, S, D = 4, 2048, 512
H_CORE, HD = 4, 64          # heads per core, head dim
DHC = H_CORE * HD           # 256 dims per core
VW = H_CORE * 2 * HD        # 512: V augmented with 64 ones-columns per head
N_CORES = 8

_CACHE = {}


def build_nc():
    nc = bacc.Bacc("TRN2", target_bir_lowering=False, debug=False,
                   num_devices=N_CORES)

    xT_d = nc.declare_dram_parameter("xT", [128, 4 * S], BF16, isOutput=False)
    wqk_d = nc.declare_dram_parameter("wqk", [128, 8 * DHC], BF16,
                                      isOutput=False)
    wv_d = nc.declare_dram_parameter("wv", [128, 4 * VW], BF16, isOutput=False)
    wo_d = nc.declare_dram_parameter("wo", [128, 2 * D], BF16, isOutput=False)
    bqk_d = nc.declare_dram_parameter("bqk", [128, 4], F32, isOutput=False)
    bvb_d = nc.declare_dram_parameter("bvb", [128, VW], F32, isOutput=False)
    out_d = nc.declare_dram_parameter("out", [S, D], F32, isOutput=True)

    NQB = 2          # q blocks of 1024
    QB = 1024
    NKT = S // 128   # 16 k tiles

    with TileContext(nc, num_cores=N_CORES) as tc:
        with (
            tc.tile_pool(name="persist", bufs=1) as pp,
            tc.tile_pool(name="pt_pool", bufs=3) as ptp,
            tc.tile_pool(name="rs_pool", bufs=2) as rsp,
            tc.tile_pool(name="ob_pool", bufs=3) as obp,
        ):
            # preload the exp ACT table before anything else: the first
            # real exp otherwise pays a ~2.7us table load that stalls the
            # whole pipeline
            scr = pp.tile([1, 8], F32, tag="scr", name="scr")
            nc.vector.memset(scr[:], 0.0)
            nc.scalar.activation(scr[:], scr[:], AF.Exp)

            # ---- load inputs (Q/K-proj operands first, merged DMAs) ----
            xTa = pp.tile([128, 4 * S], BF16, tag="xTa", name="xTa")
            wqka = pp.tile([128, 8 * DHC], BF16, tag="wqka", name="wqka")
            wva = pp.tile([128, 4 * VW], BF16, tag="wva", name="wva")
            woa = pp.tile([128, 2 * D], BF16, tag="woa", name="woa")
            bqka = pp.tile([128, 4], F32, tag="bqka", name="bqka")
            bvb = pp.tile([128, VW], F32, tag="bvb")
            nc.sync.dma_start(out=xTa[:], in_=xT_d[:])
            nc.sync.dma_start(out=wqka[:], in_=wqk_d[:])
            nc.sync.dma_start(out=bqka[:], in_=bqk_d[:])
            nc.sync.dma_start(out=bvb[:], in_=bvb_d[:])
            nc.sync.dma_start(out=wva[:], in_=wv_d[:])
            nc.sync.dma_start(out=woa[:], in_=wo_d[:])

            xT = [xTa[:, S * i:S * (i + 1)] for i in range(4)]
            wq = [wqka[:, DHC * i:DHC * (i + 1)] for i in range(4)]
            wk = [wqka[:, DHC * (4 + i):DHC * (5 + i)] for i in range(4)]
            wv = [wva[:, VW * i:VW * (i + 1)] for i in range(4)]
            wo = [woa[:, D * p:D * (p + 1)] for p in range(2)]
            bq = [bqka[:, p:p + 1] for p in range(2)]
            bk = [bqka[:, 2 + p:3 + p] for p in range(2)]

            QTh = [pp.tile([128, S], BF16, tag=f"QTh{h}", name=f"QTh{h}")
                   for h in range(H_CORE)]
            KTh = [pp.tile([128, S], BF16, tag=f"KTh{h}", name=f"KTh{h}")
                   for h in range(H_CORE)]
            OT = [pp.tile([128, S], BF16, tag=f"OT{p}", name=f"OT{p}")
                  for p in range(2)]
            V = [pp.tile([128, VW], BF16, tag=f"V{st}", name=f"V{st}")
                 for st in range(NKT)]
            for h in range(H_CORE):
                nc.vector.memset(QTh[h][64:128, :], 0.0)
                nc.vector.memset(KTh[h][64:128, :], 0.0)


            # ---- attention + priority work queue ----
            # queue items: (min_kt, cost, fn); a per-step budget of 2 is
            # drained smallest-min_kt-first into the PE slack
            with (
                tc.tile_pool(name="s_ps", bufs=2, space="PSUM") as sps,
                tc.tile_pool(name="o_ps", bufs=2, space="PSUM") as ops,
                tc.tile_pool(name="aux_ps", bufs=2, space="PSUM") as axp,
            ):
                aux = []
                fast = []       # normalize closures: jump the main queue

                def v_proj(st):
                    def run():
                        ps = axp.tile([128, 512], F32, tag="aux", name="aux")
                        for din in range(4):
                            nc.tensor.matmul(
                                ps[:, 0:VW],
                                xT[din][:, 128 * st:128 * (st + 1)],
                                wv[din][:],
                                start=(din == 0), stop=(din == 3),
                            )
                        nc.vector.tensor_tensor(
                            out=V[st][:], in0=ps[:, 0:VW], in1=bvb[:],
                            op=ALU.add)
                    return run

                def qk_proj(w_sb, b_sb, dst, wcol, st):
                    def run():
                        ps = axp.tile([128, 512], F32, tag="aux", name="aux")
                        for din in range(4):
                            nc.tensor.matmul(
                                ps[:],
                                w_sb[din][:, 128 * wcol:128 * (wcol + 1)],
                                xT[din][:, 512 * st:512 * (st + 1)],
                                start=(din == 0), stop=(din == 3),
                            )
                        for m in range(2):
                            nc.vector.tensor_scalar(
                                out=dst[2 * wcol + m][0:64,
                                                      512 * st:512 * (st + 1)],
                                in0=ps[64 * m:64 * (m + 1), :],
                                scalar1=b_sb[wcol][64 * m:64 * (m + 1), :],
                                scalar2=None, op0=ALU.add,
                            )
                    return run

                def outproj(st, pool, copy_engine=None):
                    def run():
                        ps = pool.tile([128, 512], F32, tag="aux",
                                       name="aux")
                        for p in range(2):
                            nc.tensor.matmul(
                                ps[:],
                                OT[p][:, 128 * st:128 * (st + 1)],
                                wo[p][:],
                                start=(p == 0), stop=(p == 1),
                            )
                        ob = obp.tile([128, D], F32, tag="ob", name="ob")
                        if copy_engine is None:
                            nc.vector.tensor_copy(ob[:], ps[:])
                        else:
                            copy_engine.copy(ob[:], ps[:])
                        nc.sync.dma_start(
                            out=out_d[128 * st:128 * (st + 1), :], in_=ob[:])
                    return run

                # inline: Q/K dout-tile-0 for seq cols 0-1023 (all that
                # blocks 0-1's S matmuls need for q-block 0 / kt 0-7) and
                # V[0..2]; everything else is deadline-ordered in the queue
                for st in range(2):
                    qk_proj(wq, bq, QTh, 0, st)()
                    qk_proj(wk, bk, KTh, 0, st)()
                v_proj(0)()
                v_proj(1)()
                v_proj(2)()
                # block 0 pops (one item per step, kt 1..15); deadlines:
                # V[st] by step st+1, K cols for S(kt) by step kt-1
                aux.append((1, 2, v_proj(3)))
                aux.append((2, 2, v_proj(4)))
                aux.append((3, 2, qk_proj(wk, bk, KTh, 0, 2)))
                for st in range(5, 9):
                    aux.append((st - 1, 2, v_proj(st)))
                aux.append((8, 2, qk_proj(wk, bk, KTh, 0, 3)))
                for st in range(9, NKT):
                    aux.append((st, 2, v_proj(st)))
                # block 1+: heads 2-3's projection, then Q cols for q-block 1
                for st in range(4):
                    aux.append((1, 2, qk_proj(wq, bq, QTh, 1, st)))
                    aux.append((1, 2, qk_proj(wk, bk, KTh, 1, st)))
                for st in (2, 3):
                    aux.append((1, 2, qk_proj(wq, bq, QTh, 0, st)))

                def block(h, qb):
                    p, m = divmod(h, 2)
                    r0, r1 = 64 * m, 64 * (m + 1)
                    q0 = QB * qb
                    o_acc = []

                    def s_mms(kt):
                        stile = sps.tile([128, QB], F32, tag="s", name="s")
                        for qt in range(2):
                            nc.tensor.matmul(
                                stile[:, 512 * qt:512 * (qt + 1)],
                                KTh[h][:, 128 * kt:128 * (kt + 1)],
                                QTh[h][:, q0 + 512 * qt:q0 + 512 * (qt + 1)],
                                start=True, stop=True,
                            )
                        return stile

                    def exp_pv(kt, stile):
                        pt = ptp.tile([128, QB], BF16, tag="pt", name="pt")
                        nc.scalar.activation(pt[:], stile[:], AF.Exp)
                        for qt in range(2):
                            nc.tensor.matmul(
                                o_acc[qt][:],
                                V[kt][:, 2 * HD * h:2 * HD * (h + 1)],
                                pt[:, 512 * qt:512 * (qt + 1)],
                                start=(kt == 0), stop=(kt == NKT - 1),
                            )

                    prev = s_mms(0)
                    for kt in range(1, NKT):
                        cur = s_mms(kt)
                        if not o_acc:
                            o_acc.extend(
                                ops.tile([128, 512], F32, tag="o",
                                         name="o_acc") for _ in range(2))
                        # fast queue first (normalizes are independent
                        # of everything queued), then strict FIFO: enqueue
                        # order encodes producer -> consumer program order
                        budget = 2
                        while fast and budget > 0 and kt >= 4:
                            fast.pop(0)()
                            budget -= 1
                        while (aux and aux[0][0] <= kt
                               and aux[0][1] <= budget):
                            _, c, fn = aux.pop(0)
                            fn()
                            budget -= c
                        exp_pv(kt - 1, prev)
                        prev = cur
                    exp_pv(NKT - 1, prev)

                    # free the o_acc PSUM slots with one fast DVE copy
                    # each; the slow reciprocal+multiply is deferred into
                    # the next block via the queue
                    osb = []
                    for qt in range(2):
                        t = rsp.tile([128, 512], F32, tag="osb", name="osb")
                        nc.vector.tensor_copy(t[:], o_acc[qt][:])
                        osb.append(t)

                    def normalize(qt):
                        def run():
                            # reciprocal_approx_fast mis-executes when any
                            # AP sits at base partition >= 64 (HW-verified),
                            # and a 2-input DVE op needs both SBUF inputs at
                            # the same base: bounce the replicated sums to a
                            # base-0 tile first (cheap 2x_2P copy), then
                            # approx-reciprocal + multiply wholly at base 0
                            sums = rsp.tile([HD, 512], F32, tag="sums",
                                            name="sums")
                            nc.vector.tensor_copy(
                                sums[:], osb[qt][HD:2 * HD, :])
                            recB = rsp.tile([HD, 512], F32, tag="recB",
                                            name="recB")
                            nc.vector.reciprocal_approx_fast(
                                recB[:], sums[:])
                            nc.vector.tensor_tensor(
                                out=OT[p][r0:r1,
                                          q0 + 512 * qt:q0 + 512 * (qt + 1)],
                                in0=osb[qt][0:HD, :], in1=recB[:],
                                op=ALU.mult,
                            )
                        return run
                    return [normalize(0), normalize(1)], (osb, p, r0, r1, q0)

                last_norm_parts = None
                for bi, (qb, h) in enumerate(
                        (qb, h) for qb in range(NQB)
                        for h in range(H_CORE)):
                    norms, parts = block(h, qb)
                    if (qb, h) == (NQB - 1, H_CORE - 1):
                        last_norm_parts = parts
                    else:
                        fast.extend(norms)
                    # qb0's out-projection pops two blocks after its last
                    # head so the deferred normalizes (behind the DVE
                    # reciprocal queue) are guaranteed to have landed
                    if bi == 4:
                        for st in range(8):
                            aux.append((1, 1, outproj(st, axp)))
                tail_sts = list(range(8, 16))
                leftovers = fast + [fn for _, _, fn in aux]

            # tail: drain with a deeper pool; interleave the last block's
            # normalize with the out-projections that don't depend on it
            with tc.tile_pool(name="tail_ps", bufs=4, space="PSUM") as tlp:
                for fn in leftovers:
                    fn()
                # chunk the last normalize per 128-col piece so each
                # final out-projection waits only on its own columns
                for qt, sts in ((0, tail_sts[:4]), (1, tail_sts[4:])):
                    osb, p, r0, r1, q0 = last_norm_parts
                    for j, st in enumerate(sts):
                        c0, c1 = 128 * j, 128 * (j + 1)
                        sumsC = rsp.tile([HD, 128], F32, tag="sumsC",
                                         name="sumsC")
                        nc.vector.tensor_copy(
                            sumsC[:], osb[qt][HD:2 * HD, c0:c1])
                        recC = rsp.tile([HD, 128], F32, tag="recC",
                                        name="recC")
                        nc.vector.reciprocal_approx_fast(
                            recC[:], sumsC[:])
                        nc.vector.tensor_tensor(
                            out=OT[p][r0:r1,
                                      q0 + 512 * qt + c0:q0 + 512 * qt + c1],
                            in0=osb[qt][0:HD, c0:c1], in1=recC[:],
                            op=ALU.mult,
                        )
                        outproj(st, tlp)()

    nc.compile()
    return nc


def _prep_core(x, wq, bq, wk, bk, wv, bv, wo, bo, b, g):
    hs = slice(DHC * g, DHC * (g + 1))

    def pack128(a):
        # [4*128, N] row-major -> [128, 4*N] with 128-row tiles side by side
        r, n = a.shape
        return np.ascontiguousarray(
            a.reshape(r // 128, 128, n).transpose(1, 0, 2).reshape(128, -1))

    xT = pack128(np.ascontiguousarray(x[b].T)).astype(ml_dtypes.bfloat16)
    wq_c = pack128(wq[:, hs] / 8.0)
    wk_c = pack128(wk[:, hs])
    wqk = np.concatenate([wq_c, wk_c], axis=1).astype(ml_dtypes.bfloat16)
    bq_c = (bq[hs] / 8.0).reshape(2, 128).T
    bk_c = bk[hs].reshape(2, 128).T
    bqk = np.concatenate([bq_c, bk_c], axis=1).astype(np.float32)
    wv_aug = np.zeros((D, VW), np.float32)
    bvb = np.zeros((128, VW), np.float32)
    for h in range(H_CORE):
        c0 = 2 * HD * h
        wv_aug[:, c0:c0 + HD] = wv[:, DHC * g + HD * h:DHC * g + HD * (h + 1)]
        bvb[:, c0:c0 + HD] = bv[DHC * g + HD * h:DHC * g + HD * (h + 1)][None, :]
        bvb[:, c0 + HD:c0 + 2 * HD] = 1.0
    wv_c = pack128(wv_aug).astype(ml_dtypes.bfloat16)
    wo_c = pack128(wo[hs, :]).astype(ml_dtypes.bfloat16)
    return {
        "xT": xT,
        "wqk": wqk, "bqk": bqk,
        "wv": wv_c, "bvb": bvb,
        "wo": wo_c,
    }


def kernel(x, wq, bq, wk, bk, wv, bv, wo, bo):
    x = np.asarray(x, np.float32)
    wq, bq = np.asarray(wq, np.float32), np.asarray(bq, np.float32)
    wk, bk = np.asarray(wk, np.float32), np.asarray(bk, np.float32)
    wv, bv = np.asarray(wv, np.float32), np.asarray(bv, np.float32)
    wo, bo = np.asarray(wo, np.float32), np.asarray(bo, np.float32)

    if "nc" not in _CACHE:
        _CACHE["nc"] = build_nc()
    nc = _CACHE["nc"]

    in_maps = []
    for c in range(N_CORES):
        b, g = divmod(c, 2)
        in_maps.append(_prep_core(x, wq, bq, wk, bk, wv, bv, wo, bo, b, g))

    res = run_bass_kernel_spmd(nc, in_maps, list(range(N_CORES)))

    out = np.empty((B, S, D), np.float32)
    for b in range(B):
        out[b] = (res.results[2 * b]["out"] + res.results[2 * b + 1]["out"]
                  + bo[None, :])
    return out


# revision 10
# speedup vs baseline: 1.1679x; 1.0668x over previous
"""Multi-head attention (B=4, S=2048, D=512, H=8) on 8 trn2 NeuronCores.

Sharding: core c handles batch b = c//2 and head-group g = c%2 (4 heads,
256 of the 512 model dims). Each core computes its 4 heads' attention and
a partial out-projection [2048, 512]; the host sums the two partials per
batch and adds the output bias.

Device kernel per core (all matmuls bf16 -> f32 PSUM):
  1. QKV projections from pre-transposed xT [512, 2048]:
       Q^T/K^T stored per head, zero-padded from 64 to 128 rows (on idle
       GpSimd) so the attention matmuls contract over K=128. V [128, 512]
       per seq-tile with 64 all-ones columns per head (injected via the
       bias) so the P@V matmul emits the softmax row-sum pre-replicated
       on its partitions 64-127 and runs the full M=128 array. wq/bq are
       pre-scaled by 1/8 on the host. xT arrives as four [128, 2048]
       DMAs (one per 512-q group, host-packed with the four 128-row
       contraction tiles side by side) interleaved with the weight loads
       so the first projection starts ~10us in instead of waiting for
       one monolithic transfer.
  2. Per (q-block, head), flash-style: S^T tile [128, 1024] = K_h^T.Q_h,
     exp on ScalarE (PSUM -> SBUF bf16, double-buffered, software
     pipelined), P^T accumulated into O^T [128, 512] over 16 k-tiles.
     ScalarE (exp) is the saturated engine; everything else (V
     projection, heads 2-3's Q/K projection, normalize, out-projection)
     rides a priority work queue drained into the loop's slack.
  3. Normalize per q-tile: bounce the replicated row-sums to a base-0
     tile (reciprocal_approx_fast mis-executes on APs at base partition
     >= 64, HW-verified), approximate reciprocal (~18-bit, 5x faster
     than the exact iterative divide), multiply into O^T.
  4. The last head's second q-block runs as two 512-wide sub-blocks so
     its normalize + final out-projections overlap the attention of the
     second sub-block instead of serializing in a cold tail; tail
     PSUM->SBUF copies ride the by-then-idle ScalarE.
No max-subtraction in softmax: scores are O(1) by construction, exp is
safe, and the reference softmax is shift-invariant.
"""

import numpy as np
import ml_dtypes

import concourse.bacc as bacc
import concourse.mybir as mybir
from concourse.tile import TileContext
from concourse.bass_utils import run_bass_kernel_spmd

BF16 = mybir.dt.bfloat16
F32 = mybir.dt.float32
AF = mybir.ActivationFunctionType
ALU = mybir.AluOpType

B, S, D = 4, 2048, 512
H_CORE, HD = 4, 64          # heads per core, head dim
DHC = H_CORE * HD           # 256 dims per core
VW = H_CORE * 2 * HD        # 512: V augmented with 64 ones-columns per head
N_CORES = 8

_CACHE = {}


def build_nc():
    nc = bacc.Bacc("TRN2", target_bir_lowering=False, debug=False,
                   num_devices=N_CORES)

    xT_d = nc.declare_dram_parameter("xT", [128, 4 * S], BF16, isOutput=False)
    wqk_d = nc.declare_dram_parameter("wqk", [128, 8 * DHC], BF16,
                                      isOutput=False)
    wv_d = nc.declare_dram_parameter("wv", [128, 4 * VW], BF16, isOutput=False)
    wo_d = nc.declare_dram_parameter("wo", [128, 2 * D], BF16, isOutput=False)
    bqk_d = nc.declare_dram_parameter("bqk", [128, 4], F32, isOutput=False)
    bvb_d = nc.declare_dram_parameter("bvb", [128, VW], F32, isOutput=False)
    out_d = nc.declare_dram_parameter("out", [S, D], F32, isOutput=True)

    QB = 1024
    NKT = S // 128   # 16 k tiles

    with TileContext(nc, num_cores=N_CORES) as tc:
        with (
            tc.tile_pool(name="persist", bufs=1) as pp,
            tc.tile_pool(name="pt_pool", bufs=3) as ptp,
            tc.tile_pool(name="rs_pool", bufs=2) as rsp,
            tc.tile_pool(name="ob_pool", bufs=3) as obp,
        ):
            # preload the exp ACT table before anything else: the first
            # real exp otherwise pays a ~2.7us table load that stalls the
            # whole pipeline
            scr = pp.tile([1, 8], F32, tag="scr", name="scr")
            nc.vector.memset(scr[:], 0.0)
            nc.scalar.activation(scr[:], scr[:], AF.Exp)

            # ---- load inputs; xT in four 512-q-group chunks so the
            # first Q/K projections start as soon as chunk 0 + wqk land
            xTg = [pp.tile([128, 4 * 512], BF16, tag=f"xTg{j}",
                           name=f"xTg{j}") for j in range(4)]
            wqka = pp.tile([128, 8 * DHC], BF16, tag="wqka", name="wqka")
            wva = pp.tile([128, 4 * VW], BF16, tag="wva", name="wva")
            woa = pp.tile([128, 2 * D], BF16, tag="woa", name="woa")
            bqka = pp.tile([128, 4], F32, tag="bqka", name="bqka")
            bvb = pp.tile([128, VW], F32, tag="bvb")
            nc.sync.dma_start(out=xTg[0][:], in_=xT_d[:, 0:2048])
            nc.sync.dma_start(out=wqka[:], in_=wqk_d[:])
            nc.sync.dma_start(out=bqka[:], in_=bqk_d[:])
            nc.sync.dma_start(out=xTg[1][:], in_=xT_d[:, 2048:4096])
            nc.sync.dma_start(out=wva[:], in_=wv_d[:])
            nc.sync.dma_start(out=bvb[:], in_=bvb_d[:])
            nc.sync.dma_start(out=xTg[2][:], in_=xT_d[:, 4096:6144])
            nc.sync.dma_start(out=xTg[3][:], in_=xT_d[:, 6144:8192])
            nc.sync.dma_start(out=woa[:], in_=wo_d[:])

            wq = [wqka[:, DHC * i:DHC * (i + 1)] for i in range(4)]
            wk = [wqka[:, DHC * (4 + i):DHC * (5 + i)] for i in range(4)]
            wv = [wva[:, VW * i:VW * (i + 1)] for i in range(4)]
            wo = [woa[:, D * p:D * (p + 1)] for p in range(2)]
            bq = [bqka[:, p:p + 1] for p in range(2)]
            bk = [bqka[:, 2 + p:3 + p] for p in range(2)]

            QTh = [pp.tile([128, S], BF16, tag=f"QTh{h}", name=f"QTh{h}")
                   for h in range(H_CORE)]
            KTh = [pp.tile([128, S], BF16, tag=f"KTh{h}", name=f"KTh{h}")
                   for h in range(H_CORE)]
            OT = [pp.tile([128, S], BF16, tag=f"OT{p}", name=f"OT{p}")
                  for p in range(2)]
            V = [pp.tile([128, VW], BF16, tag=f"V{st}", name=f"V{st}")
                 for st in range(NKT)]
            # zero the contraction padding on the otherwise-idle GpSimd so
            # the DVE isn't serialized on 14us of memsets during ramp-up
            for h in range(H_CORE):
                nc.gpsimd.memset(QTh[h][64:128, :], 0.0)
                nc.gpsimd.memset(KTh[h][64:128, :], 0.0)


            # ---- attention + priority work queue ----
            # queue items: (min_kt, cost, fn); a per-step budget of 2 is
            # drained smallest-min_kt-first into the PE slack
            with (
                tc.tile_pool(name="s_ps", bufs=2, space="PSUM") as sps,
                tc.tile_pool(name="o_ps", bufs=2, space="PSUM") as ops,
                tc.tile_pool(name="aux_ps", bufs=2, space="PSUM") as axp,
            ):
                aux = []
                fast = []       # normalize closures: jump the main queue

                def v_proj(st):
                    def run():
                        ps = axp.tile([128, 512], F32, tag="aux", name="aux")
                        for din in range(4):
                            nc.tensor.matmul(
                                ps[:, 0:VW],
                                xTg[st // 4][:, 512 * din + 128 * (st % 4):
                                             512 * din + 128 * (st % 4 + 1)],
                                wv[din][:],
                                start=(din == 0), stop=(din == 3),
                            )
                        nc.vector.tensor_tensor(
                            out=V[st][:], in0=ps[:, 0:VW], in1=bvb[:],
                            op=ALU.add)
                    return run

                def qk_proj(w_sb, b_sb, dst, wcol, st):
                    def run():
                        ps = axp.tile([128, 512], F32, tag="aux", name="aux")
                        for din in range(4):
                            nc.tensor.matmul(
                                ps[:],
                                w_sb[din][:, 128 * wcol:128 * (wcol + 1)],
                                xTg[st][:, 512 * din:512 * (din + 1)],
                                start=(din == 0), stop=(din == 3),
                            )
                        for m in range(2):
                            nc.vector.tensor_scalar(
                                out=dst[2 * wcol + m][0:64,
                                                      512 * st:512 * (st + 1)],
                                in0=ps[64 * m:64 * (m + 1), :],
                                scalar1=b_sb[wcol][64 * m:64 * (m + 1), :],
                                scalar2=None, op0=ALU.add,
                            )
                    return run

                def outproj(st, pool, copy_engine=None):
                    def run():
                        ps = pool.tile([128, 512], F32, tag="aux",
                                       name="aux")
                        for p in range(2):
                            nc.tensor.matmul(
                                ps[:],
                                OT[p][:, 128 * st:128 * (st + 1)],
                                wo[p][:],
                                start=(p == 0), stop=(p == 1),
                            )
                        ob = obp.tile([128, D], F32, tag="ob", name="ob")
                        if copy_engine is None:
                            nc.vector.tensor_copy(ob[:], ps[:])
                        else:
                            copy_engine.copy(ob[:], ps[:])
                        nc.sync.dma_start(
                            out=out_d[128 * st:128 * (st + 1), :], in_=ob[:])
                    return run

                # inline: Q/K dout-tile-0 for seq cols 0-1023 (all that
                # blocks 0-1's S matmuls need for q-block 0 / kt 0-7) and
                # V[0..2]; everything else is deadline-ordered in the queue
                for st in range(2):
                    qk_proj(wq, bq, QTh, 0, st)()
                    qk_proj(wk, bk, KTh, 0, st)()
                v_proj(0)()
                v_proj(1)()
                v_proj(2)()
                # block 0 pops (one item per step, kt 1..15); deadlines:
                # V[st] by step st+1, K cols for S(kt) by step kt-1
                aux.append((1, 2, v_proj(3)))
                aux.append((2, 2, v_proj(4)))
                aux.append((3, 2, qk_proj(wk, bk, KTh, 0, 2)))
                for st in range(5, 9):
                    aux.append((st - 1, 2, v_proj(st)))
                aux.append((8, 2, qk_proj(wk, bk, KTh, 0, 3)))
                for st in range(9, NKT):
                    aux.append((st, 2, v_proj(st)))
                # block 1+: heads 2-3's projection, then Q cols for q-block 1
                for st in range(4):
                    aux.append((1, 2, qk_proj(wq, bq, QTh, 1, st)))
                    aux.append((1, 2, qk_proj(wk, bk, KTh, 1, st)))
                for st in (2, 3):
                    aux.append((1, 2, qk_proj(wq, bq, QTh, 0, st)))

                def block(h, q0, qw):
                    p, m = divmod(h, 2)
                    r0, r1 = 64 * m, 64 * (m + 1)
                    nqt = qw // 512
                    o_acc = []

                    def s_mms(kt):
                        stile = sps.tile([128, QB], F32, tag="s", name="s")
                        for qt in range(nqt):
                            nc.tensor.matmul(
                                stile[:, 512 * qt:512 * (qt + 1)],
                                KTh[h][:, 128 * kt:128 * (kt + 1)],
                                QTh[h][:, q0 + 512 * qt:q0 + 512 * (qt + 1)],
                                start=True, stop=True,
                            )
                        return stile

                    def exp_pv(kt, stile):
                        pt = ptp.tile([128, QB], BF16, tag="pt", name="pt")
                        nc.scalar.activation(pt[:, 0:qw], stile[:, 0:qw],
                                             AF.Exp)
                        for qt in range(nqt):
                            nc.tensor.matmul(
                                o_acc[qt][:],
                                V[kt][:, 2 * HD * h:2 * HD * (h + 1)],
                                pt[:, 512 * qt:512 * (qt + 1)],
                                start=(kt == 0), stop=(kt == NKT - 1),
                            )

                    prev = s_mms(0)
                    for kt in range(1, NKT):
                        cur = s_mms(kt)
                        if not o_acc:
                            o_acc.extend(
                                ops.tile([128, 512], F32, tag="o",
                                         name="o_acc") for _ in range(nqt))
                        # fast queue first (normalizes are independent
                        # of everything queued), then strict FIFO: enqueue
                        # order encodes producer -> consumer program order
                        budget = 2
                        while fast and budget > 0 and kt >= 4:
                            fast.pop(0)()
                            budget -= 1
                        while (aux and aux[0][0] <= kt
                               and aux[0][1] <= budget):
                            _, c, fn = aux.pop(0)
                            fn()
                            budget -= c
                        exp_pv(kt - 1, prev)
                        prev = cur
                    exp_pv(NKT - 1, prev)

                    # free the o_acc PSUM slots with one fast DVE copy
                    # each; the slow normalize is deferred into the next
                    # block via the queue
                    osb = []
                    for qt in range(nqt):
                        t = rsp.tile([128, 512], F32, tag="osb", name="osb")
                        nc.vector.tensor_copy(t[:], o_acc[qt][:])
                        osb.append(t)

                    def normalize(qt):
                        def run():
                            # reciprocal_approx_fast mis-executes when any
                            # AP sits at base partition >= 64 (HW-verified)
                            # and a 2-input DVE op needs both SBUF inputs
                            # at one base: bounce the replicated sums to a
                            # base-0 tile (cheap 2x_2P copy) first
                            sums = rsp.tile([HD, 512], F32, tag="sums",
                                            name="sums")
                            nc.vector.tensor_copy(
                                sums[:], osb[qt][HD:2 * HD, :])
                            recB = rsp.tile([HD, 512], F32, tag="recB",
                                            name="recB")
                            nc.vector.reciprocal_approx_fast(
                                recB[:], sums[:])
                            nc.vector.tensor_tensor(
                                out=OT[p][r0:r1,
                                          q0 + 512 * qt:q0 + 512 * (qt + 1)],
                                in0=osb[qt][0:HD, :], in1=recB[:],
                                op=ALU.mult,
                            )
                        return run
                    return ([normalize(qt) for qt in range(nqt)],
                            (osb, p, r0, r1, q0))

                # qb0 h0-h3, qb1 h0-h2 as full 1024-wide blocks
                seq = [(0, h) for h in range(H_CORE)] + \
                      [(1, h) for h in range(H_CORE - 1)]
                for bi, (qb, h) in enumerate(seq):
                    norms, parts = block(h, QB * qb, QB)
                    fast.extend(norms)
                    # qb0's out-projection pops two blocks after its last
                    # head so the deferred normalizes (behind the DVE
                    # queue) are guaranteed to have landed
                    if bi == 4:
                        for st in range(8):
                            aux.append((1, 1, outproj(st, axp)))
                # last head, first 512-q sub-block: its normalize and the
                # out-projections it unblocks (st 8-11) drain into the
                # second sub-block's PE slack instead of a cold tail
                normsA, _ = block(H_CORE - 1, QB, 512)
                aux.append((1, 1, normsA[0]))
                for j in range(4):
                    aux.append((3 + 2 * j, 1, outproj(8 + j, axp)))
                normsB, partsB = block(H_CORE - 1, QB + 512, 512)
                leftovers = fast + [fn for _, _, fn in aux]

            # tail: drain with a deeper pool; chunk the last normalize per
            # 128-col piece so each final out-projection waits only on its
            # own columns, and copy PSUM->SBUF on the now-idle ScalarE
            with tc.tile_pool(name="tail_ps", bufs=4, space="PSUM") as tlp:
                for fn in leftovers:
                    fn()
                osb, p, r0, r1, q0 = partsB
                for j, st in enumerate(range(12, 16)):
                    c0, c1 = 128 * j, 128 * (j + 1)
                    sumsC = rsp.tile([HD, 128], F32, tag="sumsC",
                                     name="sumsC")
                    nc.vector.tensor_copy(
                        sumsC[:], osb[0][HD:2 * HD, c0:c1])
                    recC = rsp.tile([HD, 128], F32, tag="recC",
                                    name="recC")
                    nc.vector.reciprocal_approx_fast(
                        recC[:], sumsC[:])
                    nc.vector.tensor_tensor(
                        out=OT[p][r0:r1, q0 + c0:q0 + c1],
                        in0=osb[0][0:HD, c0:c1], in1=recC[:],
                        op=ALU.mult,
                    )
                    outproj(st, tlp, copy_engine=nc.scalar)()

    nc.compile()
    return nc


def _prep_core(x, wq, bq, wk, bk, wv, bv, wo, bo, b, g):
    hs = slice(DHC * g, DHC * (g + 1))

    def pack128(a):
        # [4*128, N] row-major -> [128, 4*N] with 128-row tiles side by side
        r, n = a.shape
        return np.ascontiguousarray(
            a.reshape(r // 128, 128, n).transpose(1, 0, 2).reshape(128, -1))

    # xT: [512, 2048] -> [128, 8192] grouped by 512-q block j, then by
    # contraction tile din: col index = 2048*j + 512*din + u
    xTf = np.ascontiguousarray(x[b].T)
    xT = np.ascontiguousarray(
        xTf.reshape(4, 128, 4, 512).transpose(1, 2, 0, 3).reshape(128, 8192)
    ).astype(ml_dtypes.bfloat16)
    wq_c = pack128(wq[:, hs] / 8.0)
    wk_c = pack128(wk[:, hs])
    wqk = np.concatenate([wq_c, wk_c], axis=1).astype(ml_dtypes.bfloat16)
    bq_c = (bq[hs] / 8.0).reshape(2, 128).T
    bk_c = bk[hs].reshape(2, 128).T
    bqk = np.concatenate([bq_c, bk_c], axis=1).astype(np.float32)
    wv_aug = np.zeros((D, VW), np.float32)
    bvb = np.zeros((128, VW), np.float32)
    for h in range(H_CORE):
        c0 = 2 * HD * h
        wv_aug[:, c0:c0 + HD] = wv[:, DHC * g + HD * h:DHC * g + HD * (h + 1)]
        bvb[:, c0:c0 + HD] = bv[DHC * g + HD * h:DHC * g + HD * (h + 1)][None, :]
        bvb[:, c0 + HD:c0 + 2 * HD] = 1.0
    wv_c = pack128(wv_aug).astype(ml_dtypes.bfloat16)
    wo_c = pack128(wo[hs, :]).astype(ml_dtypes.bfloat16)
    return {
        "xT": xT,
        "wqk": wqk, "bqk": bqk,
        "wv": wv_c, "bvb": bvb,
        "wo": wo_c,
    }


def kernel(x, wq, bq, wk, bk, wv, bv, wo, bo):
    x = np.asarray(x, np.float32)
    wq, bq = np.asarray(wq, np.float32), np.asarray(bq, np.float32)
    wk, bk = np.asarray(wk, np.float32), np.asarray(bk, np.float32)
    wv, bv = np.asarray(wv, np.float32), np.asarray(bv, np.float32)
    wo, bo = np.asarray(wo, np.float32), np.asarray(bo, np.float32)

    if "nc" not in _CACHE:
        _CACHE["nc"] = build_nc()
    nc = _CACHE["nc"]

    in_maps = []
    for c in range(N_CORES):
        b, g = divmod(c, 2)
        in_maps.append(_prep_core(x, wq, bq, wk, bk, wv, bv, wo, bo, b, g))

    res = run_bass_kernel_spmd(nc, in_maps, list(range(N_CORES)))

    out = np.empty((B, S, D), np.float32)
    for b in range(B):
        out[b] = (res.results[2 * b]["out"] + res.results[2 * b + 1]["out"]
                  + bo[None, :])
    return out


# revision 16
# speedup vs baseline: 1.4734x; 1.2616x over previous
"""Multi-head attention (B=4, S=2048, D=512, H=8) on 8 trn2 NeuronCores.

Sharding: core c handles batch b = c//2 and head-group g = c%2 (4 heads,
256 of the 512 model dims). Each core computes its 4 heads' attention and
a partial out-projection [2048, 512]; the host sums the two partials per
batch and adds the output bias.

Device kernel per core (all matmuls bf16 -> f32 PSUM):
  1. QKV projections from pre-transposed xT [512, 2048], which arrives
     as four [128, 2048] DMAs (one per 512-q group, host-packed with the
     four 128-row contraction tiles side by side) interleaved with the
     weight loads so the first projection starts ~10us in. Q^T/K^T are
     stored as HEAD-PAIR tiles [128, S]: head 2p on partitions 0-63,
     head 2p+1 on 64-127 -- no zero padding. V [128, 512] per seq-tile
     with 64 all-ones columns per head (injected via the bias) so the
     P@V matmul emits the softmax row-sum pre-replicated and runs the
     full M=128 array. wq/bq are pre-scaled by 1/8 on the host.
  2. Per (512-q block, head-PAIR), flash-style: the two heads' S^T
     matmuls contract K=64 from partition bases 0 and 64, which the PE
     runs CONCURRENTLY in disjoint row-groups (tile_position is derived
     from the lhsT base partition; HW-measured 1.8x vs serial K=128
     matmuls). One exp on ScalarE covers both heads' [128, 512] scores
     (PSUM -> SBUF bf16, double-buffered); P^T accumulates into per-head
     O^T over 16 k-tiles. ScalarE (exp) is the saturated engine;
     everything else (V projection, pair 1's Q/K projection, normalize,
     out-projection) rides a priority work queue drained into the slack.
  3. Normalize per (head, block): bounce the replicated row-sums to a
     base-0 tile (reciprocal_approx_fast mis-executes on APs at base
     partition >= 64, HW-verified), approximate reciprocal (~18-bit, 5x
     faster than the exact iterative divide), multiply into O^T.
  4. Pair 0 runs all four q-blocks first, then pair 1; each pair-1
     block's normalize releases four out-projections into the next
     block's PE slack, leaving only q-block 3's four in the tail (whose
     PSUM->SBUF copies ride the by-then-idle ScalarE).
No max-subtraction in softmax: scores are O(1) by construction, exp is
safe, and the reference softmax is shift-invariant.
"""

import numpy as np
import ml_dtypes

import concourse.bacc as bacc
import concourse.mybir as mybir
from concourse.tile import TileContext
from concourse.bass_utils import run_bass_kernel_spmd

BF16 = mybir.dt.bfloat16
F32 = mybir.dt.float32
AF = mybir.ActivationFunctionType
ALU = mybir.AluOpType

B, S, D = 4, 2048, 512
H_CORE, HD = 4, 64          # heads per core, head dim
DHC = H_CORE * HD           # 256 dims per core
VW = H_CORE * 2 * HD        # 512: V augmented with 64 ones-columns per head
N_CORES = 8

_CACHE = {}


def build_nc():
    nc = bacc.Bacc("TRN2", target_bir_lowering=False, debug=False,
                   num_devices=N_CORES)

    xT_d = nc.declare_dram_parameter("xT", [128, 4 * S], BF16, isOutput=False)
    wqk_d = nc.declare_dram_parameter("wqk", [128, 8 * DHC], BF16,
                                      isOutput=False)
    wv_d = nc.declare_dram_parameter("wv", [128, 4 * VW], BF16, isOutput=False)
    wo_d = nc.declare_dram_parameter("wo", [128, 2 * D], BF16, isOutput=False)
    bqk_d = nc.declare_dram_parameter("bqk", [128, 4], F32, isOutput=False)
    bvb_d = nc.declare_dram_parameter("bvb", [128, VW], F32, isOutput=False)
    out_d = nc.declare_dram_parameter("out", [S, D], F32, isOutput=True)

    NKT = S // 128   # 16 k tiles

    with TileContext(nc, num_cores=N_CORES) as tc:
        with (
            tc.tile_pool(name="persist", bufs=1) as pp,
            tc.tile_pool(name="pt_pool", bufs=3) as ptp,
            tc.tile_pool(name="rs_pool", bufs=2) as rsp,
            tc.tile_pool(name="ob_pool", bufs=3) as obp,
        ):
            # preload the exp ACT table before anything else: the first
            # real exp otherwise pays a ~2.7us table load that stalls the
            # whole pipeline
            scr = pp.tile([1, 8], F32, tag="scr", name="scr")
            nc.vector.memset(scr[:], 0.0)
            nc.scalar.activation(scr[:], scr[:], AF.Exp)

            # ---- load inputs; xT in four 512-q-group chunks so the
            # first Q/K projections start as soon as chunk 0 + wqk land
            xTg = [pp.tile([128, 4 * 512], BF16, tag=f"xTg{j}",
                           name=f"xTg{j}") for j in range(4)]
            wqka = pp.tile([128, 8 * DHC], BF16, tag="wqka", name="wqka")
            wva = pp.tile([128, 4 * VW], BF16, tag="wva", name="wva")
            woa = pp.tile([128, 2 * D], BF16, tag="woa", name="woa")
            bqka = pp.tile([128, 4], F32, tag="bqka", name="bqka")
            bvb = pp.tile([128, VW], F32, tag="bvb")
            nc.sync.dma_start(out=xTg[0][:], in_=xT_d[:, 0:2048])
            nc.sync.dma_start(out=wqka[:], in_=wqk_d[:])
            nc.sync.dma_start(out=bqka[:], in_=bqk_d[:])
            nc.sync.dma_start(out=xTg[1][:], in_=xT_d[:, 2048:4096])
            nc.sync.dma_start(out=wva[:], in_=wv_d[:])
            nc.sync.dma_start(out=bvb[:], in_=bvb_d[:])
            nc.sync.dma_start(out=xTg[2][:], in_=xT_d[:, 4096:6144])
            nc.sync.dma_start(out=xTg[3][:], in_=xT_d[:, 6144:8192])
            nc.sync.dma_start(out=woa[:], in_=wo_d[:])

            wq = [wqka[:, DHC * i:DHC * (i + 1)] for i in range(4)]
            wk = [wqka[:, DHC * (4 + i):DHC * (5 + i)] for i in range(4)]
            wv = [wva[:, VW * i:VW * (i + 1)] for i in range(4)]
            wo = [woa[:, D * p:D * (p + 1)] for p in range(2)]

            # head-pair tiles: head 2p on partitions 0-63, 2p+1 on 64-127
            QTp = [pp.tile([128, S], BF16, tag=f"QTp{p}", name=f"QTp{p}")
                   for p in range(2)]
            KTp = [pp.tile([128, S], BF16, tag=f"KTp{p}", name=f"KTp{p}")
                   for p in range(2)]
            OT = [pp.tile([128, S], BF16, tag=f"OT{p}", name=f"OT{p}")
                  for p in range(2)]
            V = [pp.tile([128, VW], BF16, tag=f"V{st}", name=f"V{st}")
                 for st in range(NKT)]

            # ---- attention + priority work queue ----
            # queue items: (min_kt, cost, fn); a per-step budget of 2 is
            # drained smallest-min_kt-first into the PE slack
            with (
                tc.tile_pool(name="s_ps", bufs=2, space="PSUM") as sps,
                tc.tile_pool(name="o_ps", bufs=2, space="PSUM") as ops,
                tc.tile_pool(name="aux_ps", bufs=2, space="PSUM") as axp,
            ):
                aux = []
                fast = []       # normalize closures: jump the main queue

                def v_proj(st):
                    def run():
                        ps = axp.tile([128, 512], F32, tag="aux", name="aux")
                        for din in range(4):
                            nc.tensor.matmul(
                                ps[:, 0:VW],
                                xTg[st // 4][:, 512 * din + 128 * (st % 4):
                                             512 * din + 128 * (st % 4 + 1)],
                                wv[din][:],
                                start=(din == 0), stop=(din == 3),
                            )
                        nc.vector.tensor_tensor(
                            out=V[st][:], in0=ps[:, 0:VW], in1=bvb[:],
                            op=ALU.add)
                    return run

                def qk_proj(w_sb, bcol, dst, p, st):
                    def run():
                        ps = axp.tile([128, 512], F32, tag="aux", name="aux")
                        for din in range(4):
                            nc.tensor.matmul(
                                ps[:],
                                w_sb[din][:, 128 * p:128 * (p + 1)],
                                xTg[st][:, 512 * din:512 * (din + 1)],
                                start=(din == 0), stop=(din == 3),
                            )
                        nc.vector.tensor_scalar(
                            out=dst[p][:, 512 * st:512 * (st + 1)],
                            in0=ps[:], scalar1=bcol, scalar2=None,
                            op0=ALU.add,
                        )
                    return run

                def qq(p, st):
                    return qk_proj(wq, bqka[:, p:p + 1], QTp, p, st)

                def qk(p, st):
                    return qk_proj(wk, bqka[:, 2 + p:3 + p], KTp, p, st)

                def outproj(st, pool, copy_engine=None):
                    def run():
                        ps = pool.tile([128, 512], F32, tag="aux",
                                       name="aux")
                        # guard matmul: reads the freshly-normalized OT[1]
                        # slice as the MOVING operand, so the DVE-complete
                        # wait sits on this matmul and stalls the PE queue.
                        # Without it the real matmuls' LDWEIGHTS (which
                        # read OT as the stationary operand and carry no
                        # wait -- the compiler keeps the merged wait on the
                        # matmul) front-run the deferred normalize and load
                        # stale O^T (HW-observed first-run NaNs).
                        nc.tensor.matmul(
                            ps[0:1, 0:128], woa[:, 0:1],
                            OT[1][:, 128 * st:128 * (st + 1)],
                            start=True, stop=True, skip_group_check=True,
                        )
                        for p in range(2):
                            nc.tensor.matmul(
                                ps[:],
                                OT[p][:, 128 * st:128 * (st + 1)],
                                wo[p][:],
                                start=(p == 0), stop=(p == 1),
                                skip_group_check=True,
                            )
                        ob = obp.tile([128, D], F32, tag="ob", name="ob")
                        if copy_engine is None:
                            nc.vector.tensor_copy(ob[:], ps[:])
                        else:
                            copy_engine.copy(ob[:], ps[:])
                        nc.sync.dma_start(
                            out=out_d[128 * st:128 * (st + 1), :], in_=ob[:])
                    return run

                def block(p, qj):
                    q0 = 512 * qj
                    o_acc = []

                    def s_mms(kt):
                        # two K=64 matmuls from partition bases 0 / 64:
                        # disjoint PE row-groups, run concurrently
                        stile = sps.tile([128, 1024], F32, tag="s", name="s")
                        for m in range(2):
                            r = slice(64 * m, 64 * (m + 1))
                            nc.tensor.matmul(
                                stile[:, 512 * m:512 * (m + 1)],
                                KTp[p][r, 128 * kt:128 * (kt + 1)],
                                QTp[p][r, q0:q0 + 512],
                                start=True, stop=True,
                            )
                        return stile

                    def exp_pv(kt, stile):
                        pt = ptp.tile([128, 1024], BF16, tag="pt", name="pt")
                        nc.scalar.activation(pt[:], stile[:], AF.Exp)
                        for m in range(2):
                            h = 2 * p + m
                            nc.tensor.matmul(
                                o_acc[m][:],
                                V[kt][:, 128 * h:128 * (h + 1)],
                                pt[:, 512 * m:512 * (m + 1)],
                                start=(kt == 0), stop=(kt == NKT - 1),
                            )

                    prev = s_mms(0)
                    for kt in range(1, NKT):
                        cur = s_mms(kt)
                        if not o_acc:
                            o_acc.extend(
                                ops.tile([128, 512], F32, tag="o",
                                         name="o_acc") for _ in range(2))
                        # fast queue first (normalizes are independent
                        # of everything queued), then strict FIFO: enqueue
                        # order encodes producer -> consumer program order
                        budget = 2
                        while fast and budget > 0 and kt >= 4:
                            fast.pop(0)()
                            budget -= 1
                        while (aux and aux[0][0] <= kt
                               and aux[0][1] <= budget):
                            _, c, fn = aux.pop(0)
                            fn()
                            budget -= c
                        exp_pv(kt - 1, prev)
                        prev = cur
                    exp_pv(NKT - 1, prev)

                    # free the o_acc PSUM slots with one fast DVE copy
                    # each; the slow normalize is deferred into the next
                    # block via the queue
                    osb = []
                    for m in range(2):
                        t = rsp.tile([128, 512], F32, tag="osb", name="osb")
                        nc.vector.tensor_copy(t[:], o_acc[m][:])
                        osb.append(t)

                    def normalize(m):
                        def run():
                            # reciprocal_approx_fast mis-executes when any
                            # AP sits at base partition >= 64 (HW-verified)
                            # and a 2-input DVE op needs both SBUF inputs
                            # at one base: bounce the replicated sums to a
                            # base-0 tile (cheap 2x_2P copy) first
                            sums = rsp.tile([HD, 512], F32, tag="sums",
                                            name="sums")
                            nc.vector.tensor_copy(
                                sums[:], osb[m][HD:2 * HD, :])
                            recB = rsp.tile([HD, 512], F32, tag="recB",
                                            name="recB")
                            nc.vector.reciprocal_approx_fast(
                                recB[:], sums[:])
                            nc.vector.tensor_tensor(
                                out=OT[p][64 * m:64 * (m + 1), q0:q0 + 512],
                                in0=osb[m][0:HD, :], in1=recB[:],
                                op=ALU.mult,
                            )
                        return run
                    return [normalize(0), normalize(1)]

                # inline: pair 0's Q q-cols 0-1023, K seq-cols 0-1023,
                # V0-2. Q st1 must NOT ride the queue: a deadline-14 pop
                # lands its DVE write ~1 step before block (0,1)'s S
                # matmuls read it, which races on cold first runs
                qq(0, 0)()
                qk(0, 0)()
                qq(0, 1)()
                qk(0, 1)()
                for st in range(3):
                    v_proj(st)()

                # block (p0, qj0): remaining V + pair-0 K + Q st1
                aux += [(1, 2, v_proj(3)), (2, 2, v_proj(4)),
                        (3, 2, v_proj(5)), (4, 2, v_proj(6)),
                        (5, 2, qk(0, 2)), (5, 2, v_proj(7)),
                        (6, 2, v_proj(8)), (7, 2, v_proj(9)),
                        (8, 2, v_proj(10)), (9, 2, qk(0, 3)),
                        (9, 2, v_proj(11)), (10, 2, v_proj(12)),
                        (11, 2, v_proj(13)), (12, 2, v_proj(14)),
                        (13, 2, v_proj(15))]
                norms = block(0, 0)
                fast.extend(norms)

                # blocks 1-3: pair 0's remaining Q + prefetch pair 1
                prefetch = [[(1, 2, qq(0, 2)), (4, 2, qk(1, 0)),
                             (8, 2, qk(1, 1))],
                            [(1, 2, qq(0, 3)), (4, 2, qk(1, 2)),
                             (8, 2, qk(1, 3))],
                            [(4, 2, qq(1, 0))]]
                for qj in range(1, 4):
                    aux += prefetch[qj - 1]
                    fast.extend(block(0, qj))

                # pair 1: out-projections trail their normalizes by two
                # full blocks -- the PE's 64-deep reorder window can hoist
                # an out-projection's LDWEIGHTS (which reads O^T) ahead of
                # in-flight matmuls, racing a normalize that completed
                # fewer than ~a block earlier (HW-observed NaNs with a
                # 2-step gap)
                for qj in range(4):
                    if qj < 3:
                        aux.append((1, 2, qq(1, qj + 1)))
                    if qj >= 2:
                        # pop late (kt>=8): the post-boundary DVE burst
                        # (osb copies + deferred normalizes) has drained
                        # by mid-block, so the guard matmul's wait is
                        # already satisfied and costs no PE stall
                        st0 = 4 * (qj - 2)
                        aux += [(8, 1, outproj(st0 + j, axp))
                                for j in range(4)]
                    norms = block(1, qj)
                    if qj < 3:
                        fast.extend(norms)
                leftovers = fast + [fn for _, _, fn in aux]

            # tail: q-blocks 2-3's out-projections + q-block 3's
            # normalize; PSUM->SBUF copies ride the now-idle ScalarE
            with tc.tile_pool(name="tail_ps", bufs=4, space="PSUM") as tlp:
                for fn in leftovers:
                    fn()
                for fn in norms:
                    fn()
                for st in range(8, 16):
                    outproj(st, tlp, copy_engine=nc.scalar)()

    nc.compile()
    return nc


def _prep_core(x, wq, bq, wk, bk, wv, bv, wo, bo, b, g):
    hs = slice(DHC * g, DHC * (g + 1))

    def pack128(a):
        # [4*128, N] row-major -> [128, 4*N] with 128-row tiles side by side
        r, n = a.shape
        return np.ascontiguousarray(
            a.reshape(r // 128, 128, n).transpose(1, 0, 2).reshape(128, -1))

    # xT: [512, 2048] -> [128, 8192] grouped by 512-q block j, then by
    # contraction tile din: col index = 2048*j + 512*din + u
    xTf = np.ascontiguousarray(x[b].T)
    xT = np.ascontiguousarray(
        xTf.reshape(4, 128, 4, 512).transpose(1, 2, 0, 3).reshape(128, 8192)
    ).astype(ml_dtypes.bfloat16)
    wq_c = pack128(wq[:, hs] / 8.0)
    wk_c = pack128(wk[:, hs])
    wqk = np.concatenate([wq_c, wk_c], axis=1).astype(ml_dtypes.bfloat16)
    bq_c = (bq[hs] / 8.0).reshape(2, 128).T
    bk_c = bk[hs].reshape(2, 128).T
    bqk = np.concatenate([bq_c, bk_c], axis=1).astype(np.float32)
    wv_aug = np.zeros((D, VW), np.float32)
    bvb = np.zeros((128, VW), np.float32)
    for h in range(H_CORE):
        c0 = 2 * HD * h
        wv_aug[:, c0:c0 + HD] = wv[:, DHC * g + HD * h:DHC * g + HD * (h + 1)]
        bvb[:, c0:c0 + HD] = bv[DHC * g + HD * h:DHC * g + HD * (h + 1)][None, :]
        bvb[:, c0 + HD:c0 + 2 * HD] = 1.0
    wv_c = pack128(wv_aug).astype(ml_dtypes.bfloat16)
    wo_c = pack128(wo[hs, :]).astype(ml_dtypes.bfloat16)
    return {
        "xT": xT,
        "wqk": wqk, "bqk": bqk,
        "wv": wv_c, "bvb": bvb,
        "wo": wo_c,
    }


def kernel(x, wq, bq, wk, bk, wv, bv, wo, bo):
    x = np.asarray(x, np.float32)
    wq, bq = np.asarray(wq, np.float32), np.asarray(bq, np.float32)
    wk, bk = np.asarray(wk, np.float32), np.asarray(bk, np.float32)
    wv, bv = np.asarray(wv, np.float32), np.asarray(bv, np.float32)
    wo, bo = np.asarray(wo, np.float32), np.asarray(bo, np.float32)

    if "nc" not in _CACHE:
        _CACHE["nc"] = build_nc()
    nc = _CACHE["nc"]

    in_maps = []
    for c in range(N_CORES):
        b, g = divmod(c, 2)
        in_maps.append(_prep_core(x, wq, bq, wk, bk, wv, bv, wo, bo, b, g))

    res = run_bass_kernel_spmd(nc, in_maps, list(range(N_CORES)))

    out = np.empty((B, S, D), np.float32)
    for b in range(B):
        out[b] = (res.results[2 * b]["out"] + res.results[2 * b + 1]["out"]
                  + bo[None, :])
    return out


# revision 18
# speedup vs baseline: 1.5207x; 1.0321x over previous
"""Multi-head attention (B=4, S=2048, D=512, H=8) on 8 trn2 NeuronCores.

Sharding: core c handles batch b = c//2 and head-group g = c%2 (4 heads,
256 of the 512 model dims). Each core computes its 4 heads' attention and
a partial out-projection [2048, 512]; the host sums the two partials per
batch and adds the output bias.

Device kernel per core (all matmuls bf16 -> f32 PSUM):
  1. QKV projections from pre-transposed xT [512, 2048], which arrives
     as four [128, 2048] DMAs (one per 512-q group, host-packed with the
     four 128-row contraction tiles side by side) interleaved with the
     weight loads so the first projection starts ~10us in. Q^T/K^T are
     stored as HEAD-PAIR tiles [128, S]: head 2p on partitions 0-63,
     head 2p+1 on 64-127 -- no zero padding. V [128, 512] per seq-tile
     with 64 all-ones columns per head (injected via the bias) so the
     P@V matmul emits the softmax row-sum pre-replicated and runs the
     full M=128 array. wq/bq are pre-scaled by 1/8 on the host.
  2. Per (512-q block, head-PAIR), flash-style: the two heads' S^T
     matmuls contract K=64 from partition bases 0 and 64, which the PE
     runs CONCURRENTLY in disjoint row-groups (tile_position is derived
     from the lhsT base partition; HW-measured 1.8x vs serial K=128
     matmuls). One exp on ScalarE covers both heads' [128, 512] scores
     (PSUM -> SBUF bf16, double-buffered); P^T accumulates into per-head
     O^T over 16 k-tiles. ScalarE (exp) is the saturated engine;
     everything else (V projection, pair 1's Q/K projection, normalize,
     out-projection) rides a priority work queue drained into the slack.
  3. Normalize per (head, block): bounce the replicated row-sums to a
     base-0 tile (reciprocal_approx_fast mis-executes on APs at base
     partition >= 64, HW-verified), approximate reciprocal (~18-bit, 5x
     faster than the exact iterative divide), multiply into O^T.
  4. Pair 0 runs all four q-blocks first, then pair 1; each pair-1
     block's normalize releases four out-projections into the next
     block's PE slack, leaving only q-block 3's four in the tail (whose
     PSUM->SBUF copies ride the by-then-idle ScalarE).
No max-subtraction in softmax: scores are O(1) by construction, exp is
safe, and the reference softmax is shift-invariant.
"""

import numpy as np
import ml_dtypes

import concourse.bacc as bacc
import concourse.mybir as mybir
from concourse.tile import TileContext
from concourse.bass_utils import run_bass_kernel_spmd

BF16 = mybir.dt.bfloat16
F32 = mybir.dt.float32
AF = mybir.ActivationFunctionType
ALU = mybir.AluOpType

B, S, D = 4, 2048, 512
H_CORE, HD = 4, 64          # heads per core, head dim
DHC = H_CORE * HD           # 256 dims per core
VW = H_CORE * 2 * HD        # 512: V augmented with 64 ones-columns per head
N_CORES = 8

_CACHE = {}


def build_nc():
    nc = bacc.Bacc("TRN2", target_bir_lowering=False, debug=False,
                   num_devices=N_CORES)

    xT_d = nc.declare_dram_parameter("xT", [128, 4 * S], BF16, isOutput=False)
    wq_d = nc.declare_dram_parameter("wqa", [128, 4 * DHC], BF16,
                                     isOutput=False)
    wk_d = nc.declare_dram_parameter("wka", [128, 4 * DHC], BF16,
                                     isOutput=False)
    wv_d = nc.declare_dram_parameter("wv", [128, 4 * DHC], BF16, isOutput=False)
    wo_d = nc.declare_dram_parameter("wo", [128, 2 * D], BF16, isOutput=False)
    bqk_d = nc.declare_dram_parameter("bqk", [128, 4], F32, isOutput=False)
    bvb_d = nc.declare_dram_parameter("bvb", [128, DHC], F32, isOutput=False)
    out_d = nc.declare_dram_parameter("out", [S, D], F32, isOutput=True)

    NKT = S // 128   # 16 k tiles

    with TileContext(nc, num_cores=N_CORES) as tc:
        with (
            tc.tile_pool(name="persist", bufs=1) as pp,
            tc.tile_pool(name="pt_pool", bufs=3) as ptp,
            tc.tile_pool(name="rs_pool", bufs=2) as rsp,
            tc.tile_pool(name="ob_pool", bufs=3) as obp,
        ):
            # preload the exp ACT table before anything else: the first
            # real exp otherwise pays a ~2.7us table load that stalls the
            # whole pipeline
            scr = pp.tile([1, 8], F32, tag="scr", name="scr")
            nc.vector.memset(scr[:], 0.0)
            nc.scalar.activation(scr[:], scr[:], AF.Exp)

            # ---- load inputs; xT in four 512-q-group chunks so the
            # first Q/K projections start as soon as chunk 0 + wqk land
            xTg = [pp.tile([128, 4 * 512], BF16, tag=f"xTg{j}",
                           name=f"xTg{j}") for j in range(4)]
            wqa = pp.tile([128, 4 * DHC], BF16, tag="wqa", name="wqa")
            wka = pp.tile([128, 4 * DHC], BF16, tag="wka", name="wka")
            wva = pp.tile([128, 4 * DHC], BF16, tag="wva", name="wva")
            woa = pp.tile([128, 2 * D], BF16, tag="woa", name="woa")
            bqka = pp.tile([128, 4], F32, tag="bqka", name="bqka")
            bvb = pp.tile([128, DHC], F32, tag="bvb")
            nc.sync.dma_start(out=bqka[:], in_=bqk_d[:])
            nc.sync.dma_start(out=xTg[0][:], in_=xT_d[:, 0:2048])
            nc.sync.dma_start(out=wqa[:], in_=wq_d[:])
            nc.sync.dma_start(out=wka[:], in_=wk_d[:])
            nc.sync.dma_start(out=xTg[1][:], in_=xT_d[:, 2048:4096])
            nc.sync.dma_start(out=wva[:], in_=wv_d[:])
            nc.sync.dma_start(out=bvb[:], in_=bvb_d[:])
            nc.sync.dma_start(out=xTg[2][:], in_=xT_d[:, 4096:6144])
            nc.sync.dma_start(out=xTg[3][:], in_=xT_d[:, 6144:8192])
            nc.sync.dma_start(out=woa[:], in_=wo_d[:])

            wq = [wqa[:, DHC * i:DHC * (i + 1)] for i in range(4)]
            wk = [wka[:, DHC * i:DHC * (i + 1)] for i in range(4)]
            wv = [wva[:, DHC * i:DHC * (i + 1)] for i in range(4)]
            wo = [woa[:, D * p:D * (p + 1)] for p in range(2)]

            # head-pair tiles: head 2p on partitions 0-63, 2p+1 on 64-127
            QTp = [pp.tile([128, S], BF16, tag=f"QTp{p}", name=f"QTp{p}")
                   for p in range(2)]
            KTp = [pp.tile([128, S], BF16, tag=f"KTp{p}", name=f"KTp{p}")
                   for p in range(2)]
            OT = [pp.tile([128, S], BF16, tag=f"OT{p}", name=f"OT{p}")
                  for p in range(2)]
            # V: [128, 4, 128] = per head [64 value cols | 64 ones
            # cols]. The ones are written once up front (idle GpSimd) and
            # the bias-add scatters the projected values via a 2-free-dim
            # AP, so the V projection matmuls only compute the 256 value
            # columns; head h's PV stationary operand V[:, h, :] then
            # emits the softmax row-sums on o_acc partitions 64-127.
            V = [pp.tile([128, 4, 2 * HD], BF16, tag=f"V{st}", name=f"V{st}")
                 for st in range(NKT)]
            for st in range(NKT):
                nc.gpsimd.memset(V[st][:, 0:4, HD:2 * HD], 1.0)

            # ---- attention + priority work queue ----
            # queue items: (min_kt, cost, fn); a per-step budget of 2 is
            # drained smallest-min_kt-first into the PE slack
            with (
                tc.tile_pool(name="s_ps", bufs=2, space="PSUM") as sps,
                tc.tile_pool(name="o_ps", bufs=2, space="PSUM") as ops,
                tc.tile_pool(name="aux_ps", bufs=2, space="PSUM") as axp,
            ):
                aux = []
                fast = []       # normalize closures: jump the main queue

                def v_proj(st):
                    def run():
                        ps = axp.tile([128, 512], F32, tag="aux", name="aux")
                        for din in range(4):
                            nc.tensor.matmul(
                                ps[:, 0:DHC],
                                xTg[st // 4][:, 512 * din + 128 * (st % 4):
                                             512 * din + 128 * (st % 4 + 1)],
                                wv[din][:],
                                start=(din == 0), stop=(din == 3),
                            )
                        nc.vector.tensor_tensor(
                            out=V[st][:, 0:4, 0:HD], in0=ps[:, 0:DHC],
                            in1=bvb[:], op=ALU.add)
                    return run

                def qk_proj(w_sb, bcol, dst, p, st):
                    def run():
                        ps = axp.tile([128, 512], F32, tag="aux", name="aux")
                        for din in range(4):
                            nc.tensor.matmul(
                                ps[:],
                                w_sb[din][:, 128 * p:128 * (p + 1)],
                                xTg[st][:, 512 * din:512 * (din + 1)],
                                start=(din == 0), stop=(din == 3),
                            )
                        nc.vector.tensor_scalar(
                            out=dst[p][:, 512 * st:512 * (st + 1)],
                            in0=ps[:], scalar1=bcol, scalar2=None,
                            op0=ALU.add,
                        )
                    return run

                def qq(p, st):
                    return qk_proj(wq, bqka[:, p:p + 1], QTp, p, st)

                def qk(p, st):
                    return qk_proj(wk, bqka[:, 2 + p:3 + p], KTp, p, st)

                def outproj(st, pool, copy_engine=None):
                    def run():
                        ps = pool.tile([128, 512], F32, tag="aux",
                                       name="aux")
                        # guard matmul: reads the freshly-normalized OT[1]
                        # slice as the MOVING operand, so the DVE-complete
                        # wait sits on this matmul and stalls the PE queue.
                        # Without it the real matmuls' LDWEIGHTS (which
                        # read OT as the stationary operand and carry no
                        # wait -- the compiler keeps the merged wait on the
                        # matmul) front-run the deferred normalize and load
                        # stale O^T (HW-observed first-run NaNs).
                        nc.tensor.matmul(
                            ps[0:1, 0:8], woa[:, 0:1],
                            OT[1][:, 128 * st:128 * st + 8],
                            start=True, stop=True, skip_group_check=True,
                        )
                        for p in range(2):
                            nc.tensor.matmul(
                                ps[:],
                                OT[p][:, 128 * st:128 * (st + 1)],
                                wo[p][:],
                                start=(p == 0), stop=(p == 1),
                                skip_group_check=True,
                            )
                        ob = obp.tile([128, D], F32, tag="ob", name="ob")
                        if copy_engine is None:
                            nc.vector.tensor_copy(ob[:], ps[:])
                        else:
                            copy_engine.copy(ob[:], ps[:])
                        nc.sync.dma_start(
                            out=out_d[128 * st:128 * (st + 1), :], in_=ob[:])
                    return run

                def block(p, qj):
                    q0 = 512 * qj
                    o_acc = []

                    def s_mms(kt):
                        # two K=64 matmuls from partition bases 0 / 64:
                        # disjoint PE row-groups, run concurrently
                        stile = sps.tile([128, 1024], F32, tag="s", name="s")
                        for m in range(2):
                            r = slice(64 * m, 64 * (m + 1))
                            nc.tensor.matmul(
                                stile[:, 512 * m:512 * (m + 1)],
                                KTp[p][r, 128 * kt:128 * (kt + 1)],
                                QTp[p][r, q0:q0 + 512],
                                start=True, stop=True,
                            )
                        return stile

                    def exp_pv(kt, stile):
                        pt = ptp.tile([128, 1024], BF16, tag="pt", name="pt")
                        nc.scalar.activation(pt[:], stile[:], AF.Exp)
                        for m in range(2):
                            h = 2 * p + m
                            nc.tensor.matmul(
                                o_acc[m][:],
                                V[kt][:, h, :],
                                pt[:, 512 * m:512 * (m + 1)],
                                start=(kt == 0), stop=(kt == NKT - 1),
                            )

                    prev = s_mms(0)
                    for kt in range(1, NKT):
                        cur = s_mms(kt)
                        if not o_acc:
                            o_acc.extend(
                                ops.tile([128, 512], F32, tag="o",
                                         name="o_acc") for _ in range(2))
                        # fast queue first (normalizes are independent
                        # of everything queued), then strict FIFO: enqueue
                        # order encodes producer -> consumer program order
                        budget = 2
                        while fast and budget > 0 and kt >= 4:
                            fast.pop(0)()
                            budget -= 1
                        while (aux and aux[0][0] <= kt
                               and aux[0][1] <= budget):
                            _, c, fn = aux.pop(0)
                            fn()
                            budget -= c
                        exp_pv(kt - 1, prev)
                        prev = cur
                    exp_pv(NKT - 1, prev)

                    # free the o_acc PSUM slots with one fast DVE copy
                    # each; the slow normalize is deferred into the next
                    # block via the queue
                    osb = []
                    for m in range(2):
                        t = rsp.tile([128, 512], F32, tag="osb", name="osb")
                        nc.vector.tensor_copy(t[:], o_acc[m][:])
                        osb.append(t)

                    def normalize(m):
                        def run():
                            # reciprocal_approx_fast mis-executes when any
                            # AP sits at base partition >= 64 (HW-verified)
                            # and a 2-input DVE op needs both SBUF inputs
                            # at one base: bounce the replicated sums to a
                            # base-0 tile (cheap 2x_2P copy) first
                            sums = rsp.tile([HD, 512], F32, tag="sums",
                                            name="sums")
                            nc.vector.tensor_copy(
                                sums[:], osb[m][HD:2 * HD, :])
                            recB = rsp.tile([HD, 512], F32, tag="recB",
                                            name="recB")
                            nc.vector.reciprocal_approx_fast(
                                recB[:], sums[:])
                            nc.vector.tensor_tensor(
                                out=OT[p][64 * m:64 * (m + 1), q0:q0 + 512],
                                in0=osb[m][0:HD, :], in1=recB[:],
                                op=ALU.mult,
                            )
                        return run
                    return [normalize(0), normalize(1)]

                # inline: pair 0's Q q-cols 0-1023, K seq-cols 0-1023,
                # V0-2. Q st1 must NOT ride the queue: a deadline-14 pop
                # lands its DVE write ~1 step before block (0,1)'s S
                # matmuls read it, which races on cold first runs
                qq(0, 0)()
                qk(0, 0)()
                qq(0, 1)()
                qk(0, 1)()
                for st in range(3):
                    v_proj(st)()

                # block (p0, qj0): remaining V + pair-0 K + Q st1
                aux += [(1, 2, v_proj(3)), (2, 2, v_proj(4)),
                        (3, 2, v_proj(5)), (4, 2, v_proj(6)),
                        (5, 2, qk(0, 2)), (5, 2, v_proj(7)),
                        (6, 2, v_proj(8)), (7, 2, v_proj(9)),
                        (8, 2, v_proj(10)), (9, 2, qk(0, 3)),
                        (9, 2, v_proj(11)), (10, 2, v_proj(12)),
                        (11, 2, v_proj(13)), (12, 2, v_proj(14)),
                        (13, 2, v_proj(15))]
                norms = block(0, 0)
                fast.extend(norms)

                # blocks 1-3: pair 0's remaining Q + prefetch pair 1
                prefetch = [[(1, 2, qq(0, 2)), (4, 2, qk(1, 0)),
                             (8, 2, qk(1, 1))],
                            [(1, 2, qq(0, 3)), (4, 2, qk(1, 2)),
                             (8, 2, qk(1, 3))],
                            [(4, 2, qq(1, 0))]]
                for qj in range(1, 4):
                    aux += prefetch[qj - 1]
                    fast.extend(block(0, qj))

                # pair 1: out-projections trail their normalizes by two
                # full blocks -- the PE's 64-deep reorder window can hoist
                # an out-projection's LDWEIGHTS (which reads O^T) ahead of
                # in-flight matmuls, racing a normalize that completed
                # fewer than ~a block earlier (HW-observed NaNs with a
                # 2-step gap)
                for qj in range(4):
                    if qj < 3:
                        aux.append((1, 2, qq(1, qj + 1)))
                    if qj >= 2:
                        # pop late (kt>=8): the post-boundary DVE burst
                        # (osb copies + deferred normalizes) has drained
                        # by mid-block, so the guard matmul's wait is
                        # already satisfied and costs no PE stall
                        sts = [4 * (qj - 2) + j for j in range(4)]
                        if qj == 3:
                            sts += [8 + j for j in range(4)]
                        aux += [(8, 1, outproj(s, axp)) for s in sts]
                    norms = block(1, qj)
                    if qj < 3:
                        fast.extend(norms)
                leftovers = fast + [fn for _, _, fn in aux]

            # tail: q-blocks 2-3's out-projections + q-block 3's
            # normalize; PSUM->SBUF copies ride the now-idle ScalarE
            with tc.tile_pool(name="tail_ps", bufs=4, space="PSUM") as tlp:
                for fn in leftovers:
                    fn()
                for fn in norms:
                    fn()
                for st in range(12, 16):
                    outproj(st, tlp, copy_engine=nc.scalar)()

    nc.compile()
    return nc


def _prep_core(x, wq, bq, wk, bk, wv, bv, wo, bo, b, g):
    hs = slice(DHC * g, DHC * (g + 1))

    def pack128(a):
        # [4*128, N] row-major -> [128, 4*N] with 128-row tiles side by side
        r, n = a.shape
        return np.ascontiguousarray(
            a.reshape(r // 128, 128, n).transpose(1, 0, 2).reshape(128, -1))

    # xT: [512, 2048] -> [128, 8192] grouped by 512-q block j, then by
    # contraction tile din: col index = 2048*j + 512*din + u
    xTf = np.ascontiguousarray(x[b].T)
    xT = np.ascontiguousarray(
        xTf.reshape(4, 128, 4, 512).transpose(1, 2, 0, 3).reshape(128, 8192)
    ).astype(ml_dtypes.bfloat16)
    wq_c = pack128(wq[:, hs] / 8.0).astype(ml_dtypes.bfloat16)
    wk_c = pack128(wk[:, hs]).astype(ml_dtypes.bfloat16)
    bq_c = (bq[hs] / 8.0).reshape(2, 128).T
    bk_c = bk[hs].reshape(2, 128).T
    bqk = np.concatenate([bq_c, bk_c], axis=1).astype(np.float32)
    bvb = np.broadcast_to(bv[hs][None, :], (128, DHC)).astype(np.float32)
    wv_c = pack128(wv[:, hs]).astype(ml_dtypes.bfloat16)
    wo_c = pack128(wo[hs, :]).astype(ml_dtypes.bfloat16)
    return {
        "xT": xT,
        "wqa": wq_c, "wka": wk_c, "bqk": bqk,
        "wv": wv_c, "bvb": bvb,
        "wo": wo_c,
    }


def kernel(x, wq, bq, wk, bk, wv, bv, wo, bo):
    x = np.asarray(x, np.float32)
    wq, bq = np.asarray(wq, np.float32), np.asarray(bq, np.float32)
    wk, bk = np.asarray(wk, np.float32), np.asarray(bk, np.float32)
    wv, bv = np.asarray(wv, np.float32), np.asarray(bv, np.float32)
    wo, bo = np.asarray(wo, np.float32), np.asarray(bo, np.float32)

    if "nc" not in _CACHE:
        _CACHE["nc"] = build_nc()
    nc = _CACHE["nc"]

    in_maps = []
    for c in range(N_CORES):
        b, g = divmod(c, 2)
        in_maps.append(_prep_core(x, wq, bq, wk, bk, wv, bv, wo, bo, b, g))

    res = run_bass_kernel_spmd(nc, in_maps, list(range(N_CORES)))

    out = np.empty((B, S, D), np.float32)
    for b in range(B):
        out[b] = (res.results[2 * b]["out"] + res.results[2 * b + 1]["out"]
                  + bo[None, :])
    return out


# revision 20
# speedup vs baseline: 1.5224x; 1.0011x over previous
"""Multi-head attention (B=4, S=2048, D=512, H=8) on 8 trn2 NeuronCores.

Sharding: core c handles batch b = c//2 and head-group g = c%2 (4 heads,
256 of the 512 model dims). Each core computes its 4 heads' attention and
a partial out-projection [2048, 512]; the host sums the two partials per
batch and adds the output bias.

Device kernel per core (all matmuls bf16 -> f32 PSUM):
  1. QKV projections from pre-transposed xT [512, 2048], which arrives
     as four [128, 2048] DMAs (one per 512-q group, host-packed with the
     four 128-row contraction tiles side by side) interleaved with the
     weight loads so the first projection starts ~10us in. Q^T/K^T are
     stored as HEAD-PAIR tiles [128, S]: head 2p on partitions 0-63,
     head 2p+1 on 64-127 -- no zero padding. V [128, 512] per seq-tile
     with 64 all-ones columns per head (injected via the bias) so the
     P@V matmul emits the softmax row-sum pre-replicated and runs the
     full M=128 array. wq/bq are pre-scaled by 1/8 on the host.
  2. Per (512-q block, head-PAIR), flash-style: the two heads' S^T
     matmuls contract K=64 from partition bases 0 and 64, which the PE
     runs CONCURRENTLY in disjoint row-groups (tile_position is derived
     from the lhsT base partition; HW-measured 1.8x vs serial K=128
     matmuls). One exp on ScalarE covers both heads' [128, 512] scores
     (PSUM -> SBUF bf16, double-buffered); P^T accumulates into per-head
     O^T over 16 k-tiles. ScalarE (exp) is the saturated engine;
     everything else (V projection, pair 1's Q/K projection, normalize,
     out-projection) rides a priority work queue drained into the slack.
  3. Normalize per (head, block): bounce the replicated row-sums to a
     base-0 tile (reciprocal_approx_fast mis-executes on APs at base
     partition >= 64, HW-verified), approximate reciprocal (~18-bit, 5x
     faster than the exact iterative divide), multiply into O^T.
  4. Pair 0 runs all four q-blocks first, then pair 1; each pair-1
     block's normalize releases four out-projections into the next
     block's PE slack, leaving only q-block 3's four in the tail (whose
     PSUM->SBUF copies ride the by-then-idle ScalarE).
No max-subtraction in softmax: scores are O(1) by construction, exp is
safe, and the reference softmax is shift-invariant.
"""

import numpy as np
import ml_dtypes

import concourse.bacc as bacc
import concourse.mybir as mybir
from concourse.tile import TileContext
from concourse.bass_utils import run_bass_kernel_spmd

BF16 = mybir.dt.bfloat16
F32 = mybir.dt.float32
AF = mybir.ActivationFunctionType
ALU = mybir.AluOpType

B, S, D = 4, 2048, 512
H_CORE, HD = 4, 64          # heads per core, head dim
DHC = H_CORE * HD           # 256 dims per core
VW = H_CORE * 2 * HD        # 512: V augmented with 64 ones-columns per head
N_CORES = 8

_CACHE = {}


def build_nc():
    nc = bacc.Bacc("TRN2", target_bir_lowering=False, debug=False,
                   num_devices=N_CORES)

    xT_d = nc.declare_dram_parameter("xT", [128, 4 * S], BF16, isOutput=False)
    wq_d = nc.declare_dram_parameter("wqa", [128, 4 * DHC], BF16,
                                     isOutput=False)
    wk_d = nc.declare_dram_parameter("wka", [128, 4 * DHC], BF16,
                                     isOutput=False)
    wv_d = nc.declare_dram_parameter("wv", [128, 4 * DHC], BF16, isOutput=False)
    wo_d = nc.declare_dram_parameter("wo", [128, 2 * D], BF16, isOutput=False)
    bqk_d = nc.declare_dram_parameter("bqk", [128, 4], F32, isOutput=False)
    bvb_d = nc.declare_dram_parameter("bvb", [128, DHC], F32, isOutput=False)
    out_d = nc.declare_dram_parameter("out", [S, D], F32, isOutput=True)

    NKT = S // 128   # 16 k tiles

    with TileContext(nc, num_cores=N_CORES) as tc:
        with (
            tc.tile_pool(name="persist", bufs=1) as pp,
            tc.tile_pool(name="pt_pool", bufs=3) as ptp,
            tc.tile_pool(name="rs_pool", bufs=2) as rsp,
            tc.tile_pool(name="ob_pool", bufs=3) as obp,
        ):
            # preload the exp ACT table before anything else: the first
            # real exp otherwise pays a ~2.7us table load that stalls the
            # whole pipeline
            scr = pp.tile([1, 8], F32, tag="scr", name="scr")
            nc.vector.memset(scr[:], 0.0)
            nc.scalar.activation(scr[:], scr[:], AF.Exp)

            # ---- load inputs; xT in four 512-q-group chunks so the
            # first Q/K projections start as soon as chunk 0 + wqk land
            xTg = [pp.tile([128, 4 * 512], BF16, tag=f"xTg{j}",
                           name=f"xTg{j}") for j in range(4)]
            wqa = pp.tile([128, 4 * DHC], BF16, tag="wqa", name="wqa")
            wka = pp.tile([128, 4 * DHC], BF16, tag="wka", name="wka")
            wva = pp.tile([128, 4 * DHC], BF16, tag="wva", name="wva")
            woa = pp.tile([128, 2 * D], BF16, tag="woa", name="woa")
            bqka = pp.tile([128, 4], F32, tag="bqka", name="bqka")
            bvb = pp.tile([128, DHC], F32, tag="bvb")
            nc.sync.dma_start(out=bqka[:], in_=bqk_d[:])
            nc.sync.dma_start(out=xTg[0][:], in_=xT_d[:, 0:2048])
            nc.sync.dma_start(out=wqa[:], in_=wq_d[:])
            nc.sync.dma_start(out=wka[:], in_=wk_d[:])
            nc.sync.dma_start(out=xTg[1][:], in_=xT_d[:, 2048:4096])
            nc.sync.dma_start(out=wva[:], in_=wv_d[:])
            nc.sync.dma_start(out=bvb[:], in_=bvb_d[:])
            nc.sync.dma_start(out=xTg[2][:], in_=xT_d[:, 4096:6144])
            nc.sync.dma_start(out=xTg[3][:], in_=xT_d[:, 6144:8192])
            nc.sync.dma_start(out=woa[:], in_=wo_d[:])

            wq = [wqa[:, DHC * i:DHC * (i + 1)] for i in range(4)]
            wk = [wka[:, DHC * i:DHC * (i + 1)] for i in range(4)]
            wv = [wva[:, DHC * i:DHC * (i + 1)] for i in range(4)]
            wo = [woa[:, D * p:D * (p + 1)] for p in range(2)]

            # head-pair tiles: head 2p on partitions 0-63, 2p+1 on 64-127
            QTp = [pp.tile([128, S], BF16, tag=f"QTp{p}", name=f"QTp{p}")
                   for p in range(2)]
            KTp = [pp.tile([128, S], BF16, tag=f"KTp{p}", name=f"KTp{p}")
                   for p in range(2)]
            OT = [pp.tile([128, S], BF16, tag=f"OT{p}", name=f"OT{p}")
                  for p in range(2)]
            # V: [128, 4, 128] = per head [64 value cols | 64 ones
            # cols]. The ones are written once up front (idle GpSimd) and
            # the bias-add scatters the projected values via a 2-free-dim
            # AP, so the V projection matmuls only compute the 256 value
            # columns; head h's PV stationary operand V[:, h, :] then
            # emits the softmax row-sums on o_acc partitions 64-127.
            V = [pp.tile([128, 4, 2 * HD], BF16, tag=f"V{st}", name=f"V{st}")
                 for st in range(NKT)]
            for st in range(NKT):
                nc.gpsimd.memset(V[st][:, 0:4, HD:2 * HD], 1.0)

            # ---- attention + priority work queue ----
            # queue items: (min_kt, cost, fn); a per-step budget of 2 is
            # drained smallest-min_kt-first into the PE slack
            with (
                tc.tile_pool(name="s_ps", bufs=2, space="PSUM") as sps,
                tc.tile_pool(name="o_ps", bufs=2, space="PSUM") as ops,
                tc.tile_pool(name="aux_ps", bufs=2, space="PSUM") as axp,
            ):
                aux = []
                fast = []       # normalize closures: jump the main queue

                def v_proj(st):
                    def run():
                        ps = axp.tile([128, 512], F32, tag="aux", name="aux")
                        for din in range(4):
                            nc.tensor.matmul(
                                ps[:, 0:DHC],
                                xTg[st // 4][:, 512 * din + 128 * (st % 4):
                                             512 * din + 128 * (st % 4 + 1)],
                                wv[din][:],
                                start=(din == 0), stop=(din == 3),
                            )
                        nc.vector.tensor_tensor(
                            out=V[st][:, 0:4, 0:HD], in0=ps[:, 0:DHC],
                            in1=bvb[:], op=ALU.add)
                    return run

                def qk_proj(w_sb, bcol, dst, p, st):
                    def run():
                        ps = axp.tile([128, 512], F32, tag="aux", name="aux")
                        for din in range(4):
                            nc.tensor.matmul(
                                ps[:],
                                w_sb[din][:, 128 * p:128 * (p + 1)],
                                xTg[st][:, 512 * din:512 * (din + 1)],
                                start=(din == 0), stop=(din == 3),
                            )
                        nc.vector.tensor_scalar(
                            out=dst[p][:, 512 * st:512 * (st + 1)],
                            in0=ps[:], scalar1=bcol, scalar2=None,
                            op0=ALU.add,
                        )
                    return run

                def qq(p, st):
                    return qk_proj(wq, bqka[:, p:p + 1], QTp, p, st)

                def qk(p, st):
                    return qk_proj(wk, bqka[:, 2 + p:3 + p], KTp, p, st)

                def outproj(st, pool, copy_engine=None):
                    def run():
                        ps = pool.tile([128, 512], F32, tag="aux",
                                       name="aux")
                        # guard matmul: reads the freshly-normalized OT[1]
                        # slice as the MOVING operand, so the DVE-complete
                        # wait sits on this matmul and stalls the PE queue.
                        # Without it the real matmuls' LDWEIGHTS (which
                        # read OT as the stationary operand and carry no
                        # wait -- the compiler keeps the merged wait on the
                        # matmul) front-run the deferred normalize and load
                        # stale O^T (HW-observed first-run NaNs).
                        nc.tensor.matmul(
                            ps[0:1, 0:8], woa[:, 0:1],
                            OT[1][:, 128 * st:128 * st + 8],
                            start=True, stop=True, skip_group_check=True,
                        )
                        for p in range(2):
                            nc.tensor.matmul(
                                ps[:],
                                OT[p][:, 128 * st:128 * (st + 1)],
                                wo[p][:],
                                start=(p == 0), stop=(p == 1),
                                skip_group_check=True,
                            )
                        ob = obp.tile([128, D], F32, tag="ob", name="ob")
                        if copy_engine is None:
                            nc.vector.tensor_copy(ob[:], ps[:])
                        else:
                            copy_engine.copy(ob[:], ps[:])
                        nc.sync.dma_start(
                            out=out_d[128 * st:128 * (st + 1), :], in_=ob[:])
                    return run

                def block(p, qj):
                    q0 = 512 * qj
                    o_acc = []

                    def s_mms(kt):
                        # two K=64 matmuls from partition bases 0 / 64:
                        # disjoint PE row-groups, run concurrently
                        stile = sps.tile([128, 1024], F32, tag="s", name="s")
                        for m in range(2):
                            r = slice(64 * m, 64 * (m + 1))
                            nc.tensor.matmul(
                                stile[:, 512 * m:512 * (m + 1)],
                                KTp[p][r, 128 * kt:128 * (kt + 1)],
                                QTp[p][r, q0:q0 + 512],
                                start=True, stop=True,
                            )
                        return stile

                    def exp_pv(kt, stile):
                        pt = ptp.tile([128, 1024], BF16, tag="pt", name="pt")
                        nc.scalar.activation(pt[:], stile[:], AF.Exp)
                        for m in range(2):
                            h = 2 * p + m
                            nc.tensor.matmul(
                                o_acc[m][:],
                                V[kt][:, h, :],
                                pt[:, 512 * m:512 * (m + 1)],
                                start=(kt == 0), stop=(kt == NKT - 1),
                            )

                    prev = s_mms(0)
                    for kt in range(1, NKT):
                        cur = s_mms(kt)
                        if not o_acc:
                            o_acc.extend(
                                ops.tile([128, 512], F32, tag="o",
                                         name="o_acc") for _ in range(2))
                        # fast queue first (normalizes are independent
                        # of everything queued), then strict FIFO: enqueue
                        # order encodes producer -> consumer program order
                        budget = 2
                        while fast and budget > 0 and kt >= 4:
                            fast.pop(0)()
                            budget -= 1
                        while (aux and aux[0][0] <= kt
                               and aux[0][1] <= budget):
                            _, c, fn = aux.pop(0)
                            fn()
                            budget -= c
                        exp_pv(kt - 1, prev)
                        prev = cur
                    exp_pv(NKT - 1, prev)

                    # free the o_acc PSUM slots with one fast DVE copy
                    # each; the slow normalize is deferred into the next
                    # block via the queue
                    osb = []
                    for m in range(2):
                        t = rsp.tile([128, 512], F32, tag="osb", name="osb")
                        nc.vector.tensor_copy(t[:], o_acc[m][:])
                        osb.append(t)

                    def normalize(m):
                        def run():
                            # reciprocal_approx_fast mis-executes when any
                            # AP sits at base partition >= 64 (HW-verified)
                            # and a 2-input DVE op needs both SBUF inputs
                            # at one base: bounce the replicated sums to a
                            # base-0 tile (cheap 2x_2P copy) first
                            sums = rsp.tile([HD, 512], F32, tag="sums",
                                            name="sums")
                            nc.vector.tensor_copy(
                                sums[:], osb[m][HD:2 * HD, :])
                            recB = rsp.tile([HD, 512], F32, tag="recB",
                                            name="recB")
                            nc.vector.reciprocal_approx_fast(
                                recB[:], sums[:])
                            nc.vector.tensor_tensor(
                                out=OT[p][64 * m:64 * (m + 1), q0:q0 + 512],
                                in0=osb[m][0:HD, :], in1=recB[:],
                                op=ALU.mult,
                            )
                        return run
                    return [normalize(0), normalize(1)], (osb, q0)

                # inline: pair 0's Q q-cols 0-1023, K seq-cols 0-1023,
                # V0-2. Q st1 must NOT ride the queue: a deadline-14 pop
                # lands its DVE write ~1 step before block (0,1)'s S
                # matmuls read it, which races on cold first runs
                qq(0, 0)()
                qk(0, 0)()
                qq(0, 1)()
                qk(0, 1)()
                for st in range(3):
                    v_proj(st)()

                # block (p0, qj0): remaining V + pair-0 K + Q st1
                aux += [(1, 2, v_proj(3)), (2, 2, v_proj(4)),
                        (3, 2, v_proj(5)), (4, 2, v_proj(6)),
                        (5, 2, qk(0, 2)), (5, 2, v_proj(7)),
                        (6, 2, v_proj(8)), (7, 2, v_proj(9)),
                        (8, 2, v_proj(10)), (9, 2, qk(0, 3)),
                        (9, 2, v_proj(11)), (10, 2, v_proj(12)),
                        (11, 2, v_proj(13)), (12, 2, v_proj(14)),
                        (13, 2, v_proj(15))]
                norms, _ = block(0, 0)
                fast.extend(norms)

                # blocks 1-3: pair 0's remaining Q + prefetch pair 1
                prefetch = [[(1, 2, qq(0, 2)), (4, 2, qk(1, 0)),
                             (8, 2, qk(1, 1))],
                            [(1, 2, qq(0, 3)), (4, 2, qk(1, 2)),
                             (8, 2, qk(1, 3))],
                            [(4, 2, qq(1, 0))]]
                for qj in range(1, 4):
                    aux += prefetch[qj - 1]
                    norms, _ = block(0, qj)
                    fast.extend(norms)

                # pair 1: out-projections trail their normalizes by two
                # full blocks -- the PE's 64-deep reorder window can hoist
                # an out-projection's LDWEIGHTS (which reads O^T) ahead of
                # in-flight matmuls, racing a normalize that completed
                # fewer than ~a block earlier (HW-observed NaNs with a
                # 2-step gap)
                for qj in range(4):
                    if qj < 3:
                        aux.append((1, 2, qq(1, qj + 1)))
                    if qj >= 2:
                        # pop late (kt>=8): the post-boundary DVE burst
                        # (osb copies + deferred normalizes) has drained
                        # by mid-block, so the guard matmul's wait is
                        # already satisfied and costs no PE stall
                        sts = [4 * (qj - 2) + j for j in range(4)]
                        if qj == 3:
                            sts += [8 + j for j in range(4)]
                        aux += [(8, 1, outproj(s, axp)) for s in sts]
                    norms, parts = block(1, qj)
                    if qj < 3:
                        fast.extend(norms)
                leftovers = fast + [fn for _, _, fn in aux]

            # tail: q-blocks 2-3's out-projections + q-block 3's
            # normalize; PSUM->SBUF copies ride the now-idle ScalarE
            with tc.tile_pool(name="tail_ps", bufs=4, space="PSUM") as tlp:
                for fn in leftovers:
                    fn()
                # chunk the last block's normalize per 128-col piece so
                # each final out-projection waits only on its own columns
                osb, q0 = parts
                for j, st in enumerate(range(12, 16)):
                    c0, c1 = 128 * j, 128 * (j + 1)
                    for m in range(2):
                        sumsC = rsp.tile([HD, 128], F32, tag="sumsC",
                                         name="sumsC")
                        nc.vector.tensor_copy(
                            sumsC[:], osb[m][HD:2 * HD, c0:c1])
                        recC = rsp.tile([HD, 128], F32, tag="recC",
                                        name="recC")
                        nc.vector.reciprocal_approx_fast(recC[:], sumsC[:])
                        nc.vector.tensor_tensor(
                            out=OT[1][64 * m:64 * (m + 1),
                                      q0 + c0:q0 + c1],
                            in0=osb[m][0:HD, c0:c1], in1=recC[:],
                            op=ALU.mult,
                        )
                    outproj(st, tlp, copy_engine=nc.scalar)()

    nc.compile()
    return nc


def _prep_core(x, wq, bq, wk, bk, wv, bv, wo, bo, b, g):
    hs = slice(DHC * g, DHC * (g + 1))

    def pack128(a):
        # [4*128, N] row-major -> [128, 4*N] with 128-row tiles side by side
        r, n = a.shape
        return np.ascontiguousarray(
            a.reshape(r // 128, 128, n).transpose(1, 0, 2).reshape(128, -1))

    # xT: [512, 2048] -> [128, 8192] grouped by 512-q block j, then by
    # contraction tile din: col index = 2048*j + 512*din + u
    xTf = np.ascontiguousarray(x[b].T)
    xT = np.ascontiguousarray(
        xTf.reshape(4, 128, 4, 512).transpose(1, 2, 0, 3).reshape(128, 8192)
    ).astype(ml_dtypes.bfloat16)
    wq_c = pack128(wq[:, hs] / 8.0).astype(ml_dtypes.bfloat16)
    wk_c = pack128(wk[:, hs]).astype(ml_dtypes.bfloat16)
    bq_c = (bq[hs] / 8.0).reshape(2, 128).T
    bk_c = bk[hs].reshape(2, 128).T
    bqk = np.concatenate([bq_c, bk_c], axis=1).astype(np.float32)
    bvb = np.broadcast_to(bv[hs][None, :], (128, DHC)).astype(np.float32)
    wv_c = pack128(wv[:, hs]).astype(ml_dtypes.bfloat16)
    wo_c = pack128(wo[hs, :]).astype(ml_dtypes.bfloat16)
    return {
        "xT": xT,
        "wqa": wq_c, "wka": wk_c, "bqk": bqk,
        "wv": wv_c, "bvb": bvb,
        "wo": wo_c,
    }


def kernel(x, wq, bq, wk, bk, wv, bv, wo, bo):
    x = np.asarray(x, np.float32)
    wq, bq = np.asarray(wq, np.float32), np.asarray(bq, np.float32)
    wk, bk = np.asarray(wk, np.float32), np.asarray(bk, np.float32)
    wv, bv = np.asarray(wv, np.float32), np.asarray(bv, np.float32)
    wo, bo = np.asarray(wo, np.float32), np.asarray(bo, np.float32)

    if "nc" not in _CACHE:
        _CACHE["nc"] = build_nc()
    nc = _CACHE["nc"]

    in_maps = []
    for c in range(N_CORES):
        b, g = divmod(c, 2)
        in_maps.append(_prep_core(x, wq, bq, wk, bk, wv, bv, wo, bo, b, g))

    res = run_bass_kernel_spmd(nc, in_maps, list(range(N_CORES)))

    out = np.empty((B, S, D), np.float32)
    for b in range(B):
        out[b] = (res.results[2 * b]["out"] + res.results[2 * b + 1]["out"]
                  + bo[None, :])
    return out


# revision 21
# speedup vs baseline: 1.5300x; 1.0050x over previous
"""Multi-head attention (B=4, S=2048, D=512, H=8) on 8 trn2 NeuronCores.

Sharding: core c handles batch b = c//2 and head-group g = c%2 (4 heads,
256 of the 512 model dims). Each core computes its 4 heads' attention and
a partial out-projection [2048, 512]; the host sums the two partials per
batch and adds the output bias.

Device kernel per core (all matmuls bf16 -> f32 PSUM):
  1. QKV projections from pre-transposed xT [512, 2048], which arrives
     as four [128, 2048] DMAs (one per 512-q group, host-packed with the
     four 128-row contraction tiles side by side) interleaved with the
     weight loads so the first projection starts ~10us in. Q^T/K^T are
     stored as HEAD-PAIR tiles [128, S]: head 2p on partitions 0-63,
     head 2p+1 on 64-127 -- no zero padding. V [128, 512] per seq-tile
     with 64 all-ones columns per head (injected via the bias) so the
     P@V matmul emits the softmax row-sum pre-replicated and runs the
     full M=128 array. wq/bq are pre-scaled by 1/8 on the host.
  2. Per (512-q block, head-PAIR), flash-style: the two heads' S^T
     matmuls contract K=64 from partition bases 0 and 64, which the PE
     runs CONCURRENTLY in disjoint row-groups (tile_position is derived
     from the lhsT base partition; HW-measured 1.8x vs serial K=128
     matmuls). One exp on ScalarE covers both heads' [128, 512] scores
     (PSUM -> SBUF bf16, double-buffered); P^T accumulates into per-head
     O^T over 16 k-tiles. ScalarE (exp) is the saturated engine;
     everything else (V projection, pair 1's Q/K projection, normalize,
     out-projection) rides a priority work queue drained into the slack.
  3. Normalize per (head, block): bounce the replicated row-sums to a
     base-0 tile (reciprocal_approx_fast mis-executes on APs at base
     partition >= 64, HW-verified), approximate reciprocal (~18-bit, 5x
     faster than the exact iterative divide), multiply into O^T.
  4. Pair 0 runs all four q-blocks first, then pair 1; each pair-1
     block's normalize releases four out-projections into the next
     block's PE slack, leaving only q-block 3's four in the tail (whose
     PSUM->SBUF copies ride the by-then-idle ScalarE).
No max-subtraction in softmax: scores are O(1) by construction, exp is
safe, and the reference softmax is shift-invariant.
"""

import numpy as np
import ml_dtypes

import concourse.bacc as bacc
import concourse.mybir as mybir
from concourse.tile import TileContext
from concourse.bass_utils import run_bass_kernel_spmd

BF16 = mybir.dt.bfloat16
F32 = mybir.dt.float32
AF = mybir.ActivationFunctionType
ALU = mybir.AluOpType

B, S, D = 4, 2048, 512
H_CORE, HD = 4, 64          # heads per core, head dim
DHC = H_CORE * HD           # 256 dims per core
VW = H_CORE * 2 * HD        # 512: V augmented with 64 ones-columns per head
N_CORES = 8

_CACHE = {}


def build_nc():
    nc = bacc.Bacc("TRN2", target_bir_lowering=False, debug=False,
                   num_devices=N_CORES)

    xT_d = nc.declare_dram_parameter("xT", [128, 4 * S], BF16, isOutput=False)
    wq_d = nc.declare_dram_parameter("wqa", [128, 4 * DHC], BF16,
                                     isOutput=False)
    wk_d = nc.declare_dram_parameter("wka", [128, 4 * DHC], BF16,
                                     isOutput=False)
    wv_d = nc.declare_dram_parameter("wv", [128, 4 * DHC], BF16, isOutput=False)
    wo_d = nc.declare_dram_parameter("wo", [128, 2 * D], BF16, isOutput=False)
    bqk_d = nc.declare_dram_parameter("bqk", [128, 4], F32, isOutput=False)
    bvb_d = nc.declare_dram_parameter("bvb", [128, DHC], F32, isOutput=False)
    out_d = nc.declare_dram_parameter("out", [S, D], F32, isOutput=True)

    NKT = S // 128   # 16 k tiles

    with TileContext(nc, num_cores=N_CORES) as tc:
        with (
            tc.tile_pool(name="persist", bufs=1) as pp,
            tc.tile_pool(name="pt_pool", bufs=3) as ptp,
            tc.tile_pool(name="rs_pool", bufs=2) as rsp,
            tc.tile_pool(name="ob_pool", bufs=3) as obp,
        ):
            # preload the exp ACT table before anything else: the first
            # real exp otherwise pays a ~2.7us table load that stalls the
            # whole pipeline
            scr = pp.tile([1, 8], F32, tag="scr", name="scr")
            nc.vector.memset(scr[:], 0.0)
            nc.scalar.activation(scr[:], scr[:], AF.Exp)

            # ---- load inputs; xT in four 512-q-group chunks so the
            # first Q/K projections start as soon as chunk 0 + wqk land
            xTg = [pp.tile([128, 4 * 512], BF16, tag=f"xTg{j}",
                           name=f"xTg{j}") for j in range(4)]
            wqa = pp.tile([128, 4 * DHC], BF16, tag="wqa", name="wqa")
            wka = pp.tile([128, 4 * DHC], BF16, tag="wka", name="wka")
            wva = pp.tile([128, 4 * DHC], BF16, tag="wva", name="wva")
            woa = pp.tile([128, 2 * D], BF16, tag="woa", name="woa")
            bqka = pp.tile([128, 4], F32, tag="bqka", name="bqka")
            bvb = pp.tile([128, DHC], F32, tag="bvb")
            nc.sync.dma_start(out=bqka[:], in_=bqk_d[:])
            nc.sync.dma_start(out=xTg[0][:], in_=xT_d[:, 0:2048])
            nc.sync.dma_start(out=wqa[:], in_=wq_d[:])
            nc.sync.dma_start(out=wka[:], in_=wk_d[:])
            nc.sync.dma_start(out=xTg[1][:], in_=xT_d[:, 2048:4096])
            nc.sync.dma_start(out=wva[:], in_=wv_d[:])
            nc.sync.dma_start(out=bvb[:], in_=bvb_d[:])
            nc.sync.dma_start(out=xTg[2][:], in_=xT_d[:, 4096:6144])
            nc.sync.dma_start(out=xTg[3][:], in_=xT_d[:, 6144:8192])
            nc.sync.dma_start(out=woa[:], in_=wo_d[:])

            wq = [wqa[:, DHC * i:DHC * (i + 1)] for i in range(4)]
            wk = [wka[:, DHC * i:DHC * (i + 1)] for i in range(4)]
            wv = [wva[:, DHC * i:DHC * (i + 1)] for i in range(4)]
            wo = [woa[:, D * p:D * (p + 1)] for p in range(2)]

            # head-pair tiles: head 2p on partitions 0-63, 2p+1 on 64-127
            QTp = [pp.tile([128, S], BF16, tag=f"QTp{p}", name=f"QTp{p}")
                   for p in range(2)]
            KTp = [pp.tile([128, S], BF16, tag=f"KTp{p}", name=f"KTp{p}")
                   for p in range(2)]
            OT = [pp.tile([128, S], BF16, tag=f"OT{p}", name=f"OT{p}")
                  for p in range(2)]
            # V: [128, 4, 128] = per head [64 value cols | 64 ones
            # cols]. The ones are written once up front (idle GpSimd) and
            # the bias-add scatters the projected values via a 2-free-dim
            # AP, so the V projection matmuls only compute the 256 value
            # columns; head h's PV stationary operand V[:, h, :] then
            # emits the softmax row-sums on o_acc partitions 64-127.
            V = [pp.tile([128, 4, 2 * HD], BF16, tag=f"V{st}", name=f"V{st}")
                 for st in range(NKT)]
            for st in range(NKT):
                nc.gpsimd.memset(V[st][:, 0:4, HD:2 * HD], 1.0)

            # ---- attention + priority work queue ----
            # queue items: (min_kt, cost, fn); a per-step budget of 2 is
            # drained smallest-min_kt-first into the PE slack
            with (
                tc.tile_pool(name="s_ps", bufs=2, space="PSUM") as sps,
                tc.tile_pool(name="o_ps", bufs=2, space="PSUM") as ops,
                tc.tile_pool(name="aux_ps", bufs=2, space="PSUM") as axp,
            ):
                aux = []
                fast = []       # normalize closures: jump the main queue

                def v_proj(st):
                    def run():
                        ps = axp.tile([128, 512], F32, tag="aux", name="aux")
                        for din in range(4):
                            nc.tensor.matmul(
                                ps[:, 0:DHC],
                                xTg[st // 4][:, 512 * din + 128 * (st % 4):
                                             512 * din + 128 * (st % 4 + 1)],
                                wv[din][:],
                                start=(din == 0), stop=(din == 3),
                            )
                        nc.vector.tensor_tensor(
                            out=V[st][:, 0:4, 0:HD], in0=ps[:, 0:DHC],
                            in1=bvb[:], op=ALU.add)
                    return run

                def qk_proj(w_sb, bcol, dst, p, st):
                    def run():
                        ps = axp.tile([128, 512], F32, tag="aux", name="aux")
                        for din in range(4):
                            nc.tensor.matmul(
                                ps[:],
                                w_sb[din][:, 128 * p:128 * (p + 1)],
                                xTg[st][:, 512 * din:512 * (din + 1)],
                                start=(din == 0), stop=(din == 3),
                            )
                        nc.vector.tensor_scalar(
                            out=dst[p][:, 512 * st:512 * (st + 1)],
                            in0=ps[:], scalar1=bcol, scalar2=None,
                            op0=ALU.add,
                        )
                    return run

                def qq(p, st):
                    return qk_proj(wq, bqka[:, p:p + 1], QTp, p, st)

                def qk(p, st):
                    return qk_proj(wk, bqka[:, 2 + p:3 + p], KTp, p, st)

                def outproj(st, pool, copy_engine=None):
                    def run():
                        ps = pool.tile([128, 512], F32, tag="aux",
                                       name="aux")
                        # guard matmul: reads the freshly-normalized OT[1]
                        # slice as the MOVING operand, so the DVE-complete
                        # wait sits on this matmul and stalls the PE queue.
                        # Without it the real matmuls' LDWEIGHTS (which
                        # read OT as the stationary operand and carry no
                        # wait -- the compiler keeps the merged wait on the
                        # matmul) front-run the deferred normalize and load
                        # stale O^T (HW-observed first-run NaNs).
                        nc.tensor.matmul(
                            ps[0:1, 0:8], woa[:, 0:1],
                            OT[1][:, 128 * st:128 * st + 8],
                            start=True, stop=True, skip_group_check=True,
                        )
                        for p in range(2):
                            nc.tensor.matmul(
                                ps[:],
                                OT[p][:, 128 * st:128 * (st + 1)],
                                wo[p][:],
                                start=(p == 0), stop=(p == 1),
                                skip_group_check=True,
                            )
                        ob = obp.tile([128, D], F32, tag="ob", name="ob")
                        if copy_engine is None:
                            nc.vector.tensor_copy(ob[:], ps[:])
                        else:
                            copy_engine.copy(ob[:], ps[:])
                        nc.sync.dma_start(
                            out=out_d[128 * st:128 * (st + 1), :], in_=ob[:])
                    return run

                def block(p, qj):
                    q0 = 512 * qj
                    o_acc = []

                    def s_mms(kt):
                        # two K=64 matmuls from partition bases 0 / 64:
                        # disjoint PE row-groups, run concurrently
                        stile = sps.tile([128, 1024], F32, tag="s", name="s")
                        for m in range(2):
                            r = slice(64 * m, 64 * (m + 1))
                            nc.tensor.matmul(
                                stile[:, 512 * m:512 * (m + 1)],
                                KTp[p][r, 128 * kt:128 * (kt + 1)],
                                QTp[p][r, q0:q0 + 512],
                                start=True, stop=True,
                            )
                        return stile

                    def exp_only(stile):
                        # issue the exp BEFORE the queue drain: ScalarE is
                        # the saturated engine, so its next instruction
                        # must never sit behind queue work in emission
                        # order
                        pt = ptp.tile([128, 1024], BF16, tag="pt", name="pt")
                        nc.scalar.activation(pt[:], stile[:], AF.Exp)
                        return pt

                    def pv_only(kt, pt):
                        for m in range(2):
                            h = 2 * p + m
                            nc.tensor.matmul(
                                o_acc[m][:],
                                V[kt][:, h, :],
                                pt[:, 512 * m:512 * (m + 1)],
                                start=(kt == 0), stop=(kt == NKT - 1),
                            )

                    prev = s_mms(0)
                    for kt in range(1, NKT):
                        cur = s_mms(kt)
                        if not o_acc:
                            o_acc.extend(
                                ops.tile([128, 512], F32, tag="o",
                                         name="o_acc") for _ in range(2))
                        pt = exp_only(prev)
                        # fast queue first (normalizes are independent
                        # of everything queued), then strict FIFO: enqueue
                        # order encodes producer -> consumer program order
                        budget = 2
                        while fast and budget > 0 and kt >= 4:
                            fast.pop(0)()
                            budget -= 1
                        while (aux and aux[0][0] <= kt
                               and aux[0][1] <= budget):
                            _, c, fn = aux.pop(0)
                            fn()
                            budget -= c
                        pv_only(kt - 1, pt)
                        prev = cur
                    pv_only(NKT - 1, exp_only(prev))

                    # free the o_acc PSUM slots with one fast DVE copy
                    # each; the slow normalize is deferred into the next
                    # block via the queue
                    osb = []
                    for m in range(2):
                        t = rsp.tile([128, 512], F32, tag="osb", name="osb")
                        nc.vector.tensor_copy(t[:], o_acc[m][:])
                        osb.append(t)

                    def normalize(m):
                        def run():
                            # reciprocal_approx_fast mis-executes when any
                            # AP sits at base partition >= 64 (HW-verified)
                            # and a 2-input DVE op needs both SBUF inputs
                            # at one base: bounce the replicated sums to a
                            # base-0 tile (cheap 2x_2P copy) first
                            sums = rsp.tile([HD, 512], F32, tag="sums",
                                            name="sums")
                            nc.vector.tensor_copy(
                                sums[:], osb[m][HD:2 * HD, :])
                            recB = rsp.tile([HD, 512], F32, tag="recB",
                                            name="recB")
                            nc.vector.reciprocal_approx_fast(
                                recB[:], sums[:])
                            nc.vector.tensor_tensor(
                                out=OT[p][64 * m:64 * (m + 1), q0:q0 + 512],
                                in0=osb[m][0:HD, :], in1=recB[:],
                                op=ALU.mult,
                            )
                        return run
                    return [normalize(0), normalize(1)], (osb, q0)

                # inline: pair 0's Q q-cols 0-1023, K seq-cols 0-1023,
                # V0-2. Q st1 must NOT ride the queue: a deadline-14 pop
                # lands its DVE write ~1 step before block (0,1)'s S
                # matmuls read it, which races on cold first runs
                qq(0, 0)()
                qk(0, 0)()
                qq(0, 1)()
                qk(0, 1)()
                for st in range(3):
                    v_proj(st)()

                # block (p0, qj0): remaining V + pair-0 K + Q st1
                aux += [(1, 2, v_proj(3)), (2, 2, v_proj(4)),
                        (3, 2, v_proj(5)), (4, 2, v_proj(6)),
                        (5, 2, qk(0, 2)), (5, 2, v_proj(7)),
                        (6, 2, v_proj(8)), (7, 2, v_proj(9)),
                        (8, 2, v_proj(10)), (9, 2, qk(0, 3)),
                        (9, 2, v_proj(11)), (10, 2, v_proj(12)),
                        (11, 2, v_proj(13)), (12, 2, v_proj(14)),
                        (13, 2, v_proj(15))]
                norms, _ = block(0, 0)
                fast.extend(norms)

                # blocks 1-3: pair 0's remaining Q + prefetch pair 1
                prefetch = [[(1, 2, qq(0, 2)), (4, 2, qk(1, 0)),
                             (8, 2, qk(1, 1))],
                            [(1, 2, qq(0, 3)), (4, 2, qk(1, 2)),
                             (8, 2, qk(1, 3))],
                            [(4, 2, qq(1, 0))]]
                for qj in range(1, 4):
                    aux += prefetch[qj - 1]
                    norms, _ = block(0, qj)
                    fast.extend(norms)

                # pair 1: out-projections trail their normalizes by two
                # full blocks -- the PE's 64-deep reorder window can hoist
                # an out-projection's LDWEIGHTS (which reads O^T) ahead of
                # in-flight matmuls, racing a normalize that completed
                # fewer than ~a block earlier (HW-observed NaNs with a
                # 2-step gap)
                for qj in range(4):
                    if qj < 3:
                        aux.append((1, 2, qq(1, qj + 1)))
                    if qj >= 2:
                        # pop late (kt>=8): the post-boundary DVE burst
                        # (osb copies + deferred normalizes) has drained
                        # by mid-block, so the guard matmul's wait is
                        # already satisfied and costs no PE stall
                        sts = [4 * (qj - 2) + j for j in range(4)]
                        if qj == 3:
                            sts += [8 + j for j in range(4)]
                        aux += [(8, 1, outproj(s, axp)) for s in sts]
                    norms, parts = block(1, qj)
                    if qj < 3:
                        fast.extend(norms)
                leftovers = fast + [fn for _, _, fn in aux]

            # tail: q-blocks 2-3's out-projections + q-block 3's
            # normalize; PSUM->SBUF copies ride the now-idle ScalarE
            with tc.tile_pool(name="tail_ps", bufs=4, space="PSUM") as tlp:
                for fn in leftovers:
                    fn()
                # chunk the last block's normalize per 128-col piece so
                # each final out-projection waits only on its own columns
                osb, q0 = parts
                for j, st in enumerate(range(12, 16)):
                    c0, c1 = 128 * j, 128 * (j + 1)
                    for m in range(2):
                        sumsC = rsp.tile([HD, 128], F32, tag="sumsC",
                                         name="sumsC")
                        nc.vector.tensor_copy(
                            sumsC[:], osb[m][HD:2 * HD, c0:c1])
                        recC = rsp.tile([HD, 128], F32, tag="recC",
                                        name="recC")
                        nc.vector.reciprocal_approx_fast(recC[:], sumsC[:])
                        nc.vector.tensor_tensor(
                            out=OT[1][64 * m:64 * (m + 1),
                                      q0 + c0:q0 + c1],
                            in0=osb[m][0:HD, c0:c1], in1=recC[:],
                            op=ALU.mult,
                        )
                    outproj(st, tlp, copy_engine=nc.scalar)()

    nc.compile()
    return nc


def _prep_core(x, wq, bq, wk, bk, wv, bv, wo, bo, b, g):
    hs = slice(DHC * g, DHC * (g + 1))

    def pack128(a):
        # [4*128, N] row-major -> [128, 4*N] with 128-row tiles side by side
        r, n = a.shape
        return np.ascontiguousarray(
            a.reshape(r // 128, 128, n).transpose(1, 0, 2).reshape(128, -1))

    # xT: [512, 2048] -> [128, 8192] grouped by 512-q block j, then by
    # contraction tile din: col index = 2048*j + 512*din + u
    xTf = np.ascontiguousarray(x[b].T)
    xT = np.ascontiguousarray(
        xTf.reshape(4, 128, 4, 512).transpose(1, 2, 0, 3).reshape(128, 8192)
    ).astype(ml_dtypes.bfloat16)
    wq_c = pack128(wq[:, hs] / 8.0).astype(ml_dtypes.bfloat16)
    wk_c = pack128(wk[:, hs]).astype(ml_dtypes.bfloat16)
    bq_c = (bq[hs] / 8.0).reshape(2, 128).T
    bk_c = bk[hs].reshape(2, 128).T
    bqk = np.concatenate([bq_c, bk_c], axis=1).astype(np.float32)
    bvb = np.broadcast_to(bv[hs][None, :], (128, DHC)).astype(np.float32)
    wv_c = pack128(wv[:, hs]).astype(ml_dtypes.bfloat16)
    wo_c = pack128(wo[hs, :]).astype(ml_dtypes.bfloat16)
    return {
        "xT": xT,
        "wqa": wq_c, "wka": wk_c, "bqk": bqk,
        "wv": wv_c, "bvb": bvb,
        "wo": wo_c,
    }


def kernel(x, wq, bq, wk, bk, wv, bv, wo, bo):
    x = np.asarray(x, np.float32)
    wq, bq = np.asarray(wq, np.float32), np.asarray(bq, np.float32)
    wk, bk = np.asarray(wk, np.float32), np.asarray(bk, np.float32)
    wv, bv = np.asarray(wv, np.float32), np.asarray(bv, np.float32)
    wo, bo = np.asarray(wo, np.float32), np.asarray(bo, np.float32)

    if "nc" not in _CACHE:
        _CACHE["nc"] = build_nc()
    nc = _CACHE["nc"]

    in_maps = []
    for c in range(N_CORES):
        b, g = divmod(c, 2)
        in_maps.append(_prep_core(x, wq, bq, wk, bk, wv, bv, wo, bo, b, g))

    res = run_bass_kernel_spmd(nc, in_maps, list(range(N_CORES)))

    out = np.empty((B, S, D), np.float32)
    for b in range(B):
        out[b] = (res.results[2 * b]["out"] + res.results[2 * b + 1]["out"]
                  + bo[None, :])
    return out
